# revision 8
# baseline (speedup 1.0000x reference)
"""Multi-head attention (B=4, S=2048, D=512, H=8) on 8 Trainium2 NeuronCores.

Sharding: core c handles batch b = c//2 and query-half h = c%2 (1024 queries).
Each core computes q = (x_q @ Wq.T + bq)/8 for its queries, k/v projections
for its batch's full 2048 keys, full softmax attention for all 8 heads, and
the output projection for its query rows.  Output rows across cores are
disjoint, so there are no collectives.

The kernel is paced by the Scalar (ACT) engine: 128 exp tiles of [128,1024]
(~1.11us each) are the serial rail (~142us).  Everything else is scheduled
around keeping ACT dense:
 - inputs land via four parallel DMA queues (sync/scalar HWDGE for the q/k
   critical path, gpsimd for v/o, vector for biases) with et/blk-sliced
   weight layouts so the first exp fires ~8us in (vs ~30us serialized);
 - a dummy exp at t=0 prefires the ~2.7us ACT table load;
 - PSUM is statically partitioned (scores 4 banks double-buffered, AV 2,
   softmax denominators 1, projections/misc 1) so projection work never
   steals the score buffers that gate exp;
 - AV consumes exp tiles one kt behind the scores (no head-of-line waits on
   the PE FIFO); the two heads of a pair run col-tiled (64 cols each) in one
   [128,512] accumulator, and the denominators accumulate via four
   concurrent M=1 ones-matmuls into one bank;
 - projections are 4-matmul units spread just-in-time across the pair loops;
 - normalization: denominator rows -> partition-0 via DMA, fast-approx
   reciprocal, PE ones-broadcast to [128,512], one DVE multiply; no PE
   transposes and no DRAM bounce;
 - output projection: pairs 0-2 + bias accumulate into y_acc during pair 3,
   pair 3 is pre-normalized and added in a short tail (single K=128 matmul
   + DVE add per 128-row block), output DMA spread over three queues.
"""

import numpy as np
import ml_dtypes

B = 4
S = 2048
D = 512
H = 8
HD = 64
SQ = 1024  # queries per core
N_CORES = 8

_cache = {}


def _build():
    """Build (once) the SPMD Bass program shared by all 8 cores."""
    import concourse.bacc as bacc
    import concourse.mybir as mybir
    import concourse.tile as tile

    f32 = mybir.dt.float32
    bf16 = mybir.dt.bfloat16
    AF = mybir.ActivationFunctionType
    OP = mybir.AluOpType

    nc = bacc.Bacc("TRN2", target_bir_lowering=False, debug=False)

    # Per-core inputs (pre-sliced / transposed / cast on host).
    xqr = nc.dram_tensor("xqr", [128, 2, 4, 512], bf16, kind="ExternalInput").ap()
    keyr = nc.dram_tensor("keyr", [128, 4, 4, 512], bf16, kind="ExternalInput").ap()
    valr = nc.dram_tensor("valr", [128, 16, 4, 128], bf16, kind="ExternalInput").ap()
    wqr = nc.dram_tensor("wqr", [128, 4, 4, 128], bf16, kind="ExternalInput").ap()
    wkr = nc.dram_tensor("wkr", [128, 4, 4, 128], bf16, kind="ExternalInput").ap()
    wvr = nc.dram_tensor("wvr", [128, 4, 512], bf16, kind="ExternalInput").ap()
    wor = nc.dram_tensor("wor", [128, 4, 512], bf16, kind="ExternalInput").ap()
    bqr = nc.dram_tensor("bqr", [128, 4], f32, kind="ExternalInput").ap()
    bkr = nc.dram_tensor("bkr", [128, 4], f32, kind="ExternalInput").ap()
    bop = nc.dram_tensor("bop", [1, D], bf16, kind="ExternalInput").ap()
    y = nc.dram_tensor("y", [SQ, D], f32, kind="ExternalOutput").ap()

    with tile.TileContext(nc) as tc:
        import contextlib

        with contextlib.ExitStack() as ctx:
            const = ctx.enter_context(tc.tile_pool(name="const", bufs=1))
            io = ctx.enter_context(tc.tile_pool(name="io", bufs=1))
            acts = ctx.enter_context(tc.tile_pool(name="acts", bufs=1))
            expp = ctx.enter_context(tc.tile_pool(name="expp", bufs=8))
            work = ctx.enter_context(tc.tile_pool(name="work", bufs=2))
            psS = ctx.enter_context(tc.tile_pool(name="psS", bufs=2, space="PSUM"))
            psAV = ctx.enter_context(tc.tile_pool(name="psAV", bufs=2, space="PSUM"))
            psDen = ctx.enter_context(tc.tile_pool(name="psDen", bufs=1, space="PSUM"))
            psP = ctx.enter_context(tc.tile_pool(name="psP", bufs=1, space="PSUM"))

            # softmax denominators, head rows on partition 0
            sums = const.tile([1, H, SQ], f32)
            rcp = const.tile([1, 2, SQ], f32)

            # ---- SBUF input tiles ------------------------------------------
            wq_sb = io.tile([128, 4, 4, 128], bf16)
            wk_sb = io.tile([128, 4, 4, 128], bf16)
            wv_sb = io.tile([128, 4, 512], bf16)
            wo_sb = io.tile([128, 4, 512], bf16)
            xq_sb = io.tile([128, 2, 4, 512], bf16)
            key_sb = io.tile([128, 4, 4, 512], bf16)
            val_sb = io.tile([128, 16, 4, 128], bf16)
            bq_sb = const.tile([128, 4], f32)
            bk_sb = const.tile([128, 4], f32)
            bop_sb = const.tile([1, D], bf16)

            qT = acts.tile([128, 4, SQ], bf16)  # q^T/8, feature-major
            kT = acts.tile([128, 4, S], bf16)  # k^T, feature-major
            v_sb = acts.tile([128, 16, H, HD], bf16)  # v natural [s, (h d)]
            outT = [
                acts.tile([128, SQ], bf16, name=f"outT{i}") for i in range(4)
            ]
            y_acc = acts.tile([128, 8, 512], bf16)

            # ---- input DMAs over three parallel queues ---------------------
            # (emitted before any scalar-engine compute so the k-path DMA
            # triggers are not stuck behind the ACT table load)
            # q path (sync HWDGE ring)
            nc.sync.dma_start(wq_sb[:, 0], wqr[:, 0])
            nc.sync.dma_start(bq_sb[:], bqr[:])
            nc.sync.dma_start(xq_sb[:, 0], xqr[:, 0])
            nc.sync.dma_start(xq_sb[:, 1], xqr[:, 1])
            for et in (1, 2, 3):
                nc.sync.dma_start(wq_sb[:, et], wqr[:, et])
            # k path (scalar HWDGE ring)
            nc.scalar.dma_start(wk_sb[:, 0], wkr[:, 0])
            nc.scalar.dma_start(bk_sb[:], bkr[:])
            for blk in range(4):
                nc.scalar.dma_start(key_sb[:, blk], keyr[:, blk])
            for et in (1, 2, 3):
                nc.scalar.dma_start(wk_sb[:, et], wkr[:, et])
            # v/o path (gpsimd SWDGE)
            nc.gpsimd.dma_start(bop_sb[:], bop[:])
            nc.gpsimd.dma_start(wv_sb[:], wvr[:])
            for st in range(16):
                nc.gpsimd.dma_start(val_sb[:, st], valr[:, st])
            nc.gpsimd.dma_start(wo_sb[:], wor[:])

            # ---- constants; dummy exp prefires the ACT table load ----------
            ones_row = const.tile([1, 128], bf16)
            nc.vector.memset(ones_row[:], 1.0)
            ones_col = const.tile([128, 1], bf16)
            nc.vector.memset(ones_col[:], 1.0)
            ones_f32 = const.tile([1, 64], f32)
            nc.vector.memset(ones_f32[:], 1.0)
            dum_in = const.tile([1, 16], f32)
            nc.vector.memset(dum_in[:], 0.0)
            dum_out = const.tile([1, 16], bf16)
            nc.scalar.activation(dum_out[:], dum_in[:], AF.Exp)

            # ---- projection units (4 matmuls + one DVE op each) ------------
            def emit_qproj(et, qn):
                ps = psP.tile([128, 512], f32, tag="pp", name=f"psq{et}{qn}")
                for dc in range(4):
                    nc.tensor.matmul(
                        ps[:],
                        lhsT=wq_sb[:, et, dc, :],
                        rhs=xq_sb[:, qn, dc, :],
                        start=(dc == 0),
                        stop=(dc == 3),
                    )
                nc.vector.tensor_scalar(
                    qT[:, et, qn * 512 : (qn + 1) * 512],
                    ps[:],
                    bq_sb[:, et : et + 1],
                    0.125,
                    OP.add,
                    OP.mult,
                )

            def emit_kproj(et, blk):
                ps = psP.tile([128, 512], f32, tag="pp", name=f"psk{et}{blk}")
                for dc in range(4):
                    nc.tensor.matmul(
                        ps[:],
                        lhsT=wk_sb[:, et, dc, :],
                        rhs=key_sb[:, blk, dc, :],
                        start=(dc == 0),
                        stop=(dc == 3),
                    )
                nc.vector.tensor_scalar(
                    kT[:, et, blk * 512 : (blk + 1) * 512],
                    ps[:],
                    bk_sb[:, et : et + 1],
                    None,
                    OP.add,
                )

            def emit_vproj(st):
                ps = psP.tile([128, 512], f32, tag="pp", name=f"psv{st}")
                for dc in range(4):
                    nc.tensor.matmul(
                        ps[:],
                        lhsT=val_sb[:, st, dc, :],
                        rhs=wv_sb[:, dc, :],
                        start=(dc == 0),
                        stop=(dc == 3),
                    )
                nc.vector.tensor_copy(
                    v_sb[:, st], ps[:].rearrange("p (h d) -> p h d", h=H)
                )

            def emit_unit(u):
                kind = u[0]
                if kind == "v":
                    emit_vproj(u[1])
                elif kind == "q":
                    emit_qproj(u[1], u[2])
                elif kind == "k":
                    emit_kproj(u[1], u[2])
                elif kind == "a":
                    emit_stage_a(u[1])

            # ---- attention emitters ----------------------------------------
            def emit_scores_exp_half(hp, kt, hh, exp_tiles):
                st_t = psS.tile([128, SQ], f32, tag="st", name=f"st{hp}_{kt}_{hh}")
                for qn in range(2):
                    nc.tensor.matmul(
                        st_t[:, qn * 512 : (qn + 1) * 512],
                        lhsT=kT[64 * hh : 64 * hh + 64, hp, kt * 128 : (kt + 1) * 128],
                        rhs=qT[64 * hh : 64 * hh + 64, hp, qn * 512 : (qn + 1) * 512],
                        start=True,
                        stop=True,
                        tile_position=(64 * hh, 0),
                    )
                e = expp.tile([128, SQ], bf16, tag="exp", name=f"exp{hp}_{kt}_{hh}")
                exp_tiles[hh][kt] = e
                nc.scalar.activation(e[:], st_t[:], AF.Exp)

            def emit_av_den_half(hp, kt, hh, exp_tiles, av, den):
                e = exp_tiles[hh][kt]
                for qc in range(2):
                    nc.tensor.matmul(
                        av[qc][64 * hh : 64 * hh + 64, :],
                        lhsT=v_sb[:, kt, 2 * hp + hh, :],
                        rhs=e[:, qc * 512 : (qc + 1) * 512],
                        start=(kt == 0),
                        stop=(kt == 15),
                        tile_position=(0, 64 * hh),
                        # two col-tiled groups (one per head) share each bank;
                        # has_written tracking is per-partition, but the sim's
                        # group check is bank-granular — skip it
                        skip_group_check=True,
                    )
                for qc in range(2):
                    j = 2 * qc + hh  # den partition 32j: (hh0,qc0),(hh1,qc0),(hh0,qc1),(hh1,qc1)
                    nc.tensor.matmul(
                        den[32 * j : 32 * j + 1, :],
                        lhsT=ones_col[:, 0:1],
                        rhs=e[:, qc * 512 : (qc + 1) * 512],
                        start=(kt == 0),
                        stop=(kt == 15),
                        tile_position=(0, 32 * j),
                        skip_group_check=True,
                    )

            # ---- normalization ---------------------------------------------
            def emit_norm_factors(hp, den_sb):
                # gpsimd queue: a trigger waiting here cannot block ACTIVATEs
                for j, (hh, qc) in enumerate(((0, 0), (1, 0), (0, 1), (1, 1))):
                    nc.gpsimd.dma_start(
                        sums[0:1, 2 * hp + hh, qc * 512 : (qc + 1) * 512],
                        den_sb[32 * j : 32 * j + 1, :],
                    )
                nc.vector.reciprocal_approx_fast(
                    rcp[0:1, :, :], sums[0:1, 2 * hp : 2 * hp + 2, :]
                )

            def emit_norm_apply(hp, avsb):
                # rb psum rows 0-63 = 1/D_h0 broadcast, 64-127 = 1/D_h1
                for qc in range(2):
                    rb = psP.tile([128, 512], f32, tag="pp", name=f"rb{hp}{qc}")
                    for hh in range(2):
                        nc.tensor.matmul(
                            rb[64 * hh : 64 * hh + 64, :],
                            lhsT=ones_f32[0:1, :],
                            rhs=rcp[0:1, hh, qc * 512 : (qc + 1) * 512],
                            start=True,
                            stop=True,
                            tile_position=(0, 64 * hh),
                        )
                    nc.vector.tensor_tensor(
                        outT[hp][:, qc * 512 : (qc + 1) * 512],
                        avsb[:, qc * 512 : (qc + 1) * 512],
                        rb[:],
                        OP.mult,
                    )

            # ---- output projection stage A (pairs 0-2 + bias) --------------
            def emit_stage_a(stq):
                ps = psP.tile([128, 512], f32, tag="pp", name=f"psyA{stq}")
                for c in range(3):
                    nc.tensor.matmul(
                        ps[:],
                        lhsT=outT[c][:, stq * 128 : (stq + 1) * 128],
                        rhs=wo_sb[:, c, :],
                        start=(c == 0),
                        stop=False,
                    )
                nc.tensor.matmul(
                    ps[:], lhsT=ones_row[0:1, :], rhs=bop_sb[:], start=False, stop=True
                )
                nc.vector.tensor_copy(y_acc[:, stq, :], ps[:])

            # ---- projection / stage-A unit schedule ------------------------
            unit_sched = [[[] for _ in range(16)] for _ in range(4)]
            for st_ in range(1, 16):
                unit_sched[0][st_ - 1].append(("v", st_))
            unit_sched[0][2].append(("k", 0, 2))
            unit_sched[0][5].append(("k", 0, 3))
            unit_sched[0][10].append(("q", 1, 0))
            unit_sched[0][11].append(("q", 1, 1))
            unit_sched[0][13].append(("k", 1, 0))
            for hpp in (1, 2):
                unit_sched[hpp][0].append(("k", hpp, 1))
                unit_sched[hpp][2].append(("k", hpp, 2))
                unit_sched[hpp][4].append(("k", hpp, 3))
                unit_sched[hpp][6].append(("q", hpp + 1, 0))
                unit_sched[hpp][8].append(("q", hpp + 1, 1))
                unit_sched[hpp][10].append(("k", hpp + 1, 0))
            unit_sched[3][0].append(("k", 3, 1))
            unit_sched[3][1].append(("k", 3, 2))
            unit_sched[3][2].append(("k", 3, 3))
            for i in range(8):
                unit_sched[3][3 + i].append(("a", i))

            # ---- prologue: minimal path to the first exp -------------------
            emit_qproj(0, 0)
            emit_qproj(0, 1)
            emit_kproj(0, 0)
            emit_kproj(0, 1)
            emit_vproj(0)

            # ---- pair loop -------------------------------------------------
            exp_store = [[[None] * 16, [None] * 16] for _ in range(4)]
            pend_apply = {}
            for hp in range(4):
                av = [
                    psAV.tile([128, 512], f32, tag="av", name=f"av{hp}_{qc}")
                    for qc in range(2)
                ]
                den = psDen.tile([128, 512], f32, tag="den", name=f"den{hp}")
                # only partitions 0/32/64/96 are matmul-written; zero the rest
                # so the pair-end full-tile copy reads initialized memory
                nc.vector.memset(den[:], 0.0)
                exp_tiles = exp_store[hp]
                for kt in range(16):
                    emit_scores_exp_half(hp, kt, 0, exp_tiles)
                    if kt > 0:
                        emit_av_den_half(hp, kt - 1, 0, exp_tiles, av, den)
                    emit_scores_exp_half(hp, kt, 1, exp_tiles)
                    if kt > 0:
                        emit_av_den_half(hp, kt - 1, 1, exp_tiles, av, den)
                    if kt == 2 and (hp - 1) in pend_apply:
                        emit_norm_apply(hp - 1, pend_apply.pop(hp - 1))
                    for u in unit_sched[hp][kt]:
                        emit_unit(u)
                for hh in range(2):
                    emit_av_den_half(hp, 15, hh, exp_tiles, av, den)
                den_sb = work.tile([128, 512], f32, tag="den_sb", name=f"densb{hp}")
                nc.vector.tensor_copy(den_sb[:], den[:])
                avsb = work.tile([128, SQ], f32, tag="avsb", name=f"avsb{hp}")
                for qc in range(2):
                    nc.vector.tensor_copy(avsb[:, qc * 512 : (qc + 1) * 512], av[qc][:])
                emit_norm_factors(hp, den_sb)
                pend_apply[hp] = avsb

            # ---- tail: normalize pair 3, add its projection, store y -------
            emit_norm_apply(3, pend_apply.pop(3))
            dma_engs = [nc.sync, nc.scalar, nc.gpsimd]
            for stq in range(8):
                psb = psS.tile([128, 512], f32, tag="st", name=f"psyB{stq}")
                nc.tensor.matmul(
                    psb[:],
                    lhsT=outT[3][:, stq * 128 : (stq + 1) * 128],
                    rhs=wo_sb[:, 3, :],
                    start=True,
                    stop=True,
                )
                ysb = work.tile([128, 512], f32, tag="ysb", name=f"ysb{stq}")
                nc.vector.tensor_tensor(ysb[:], psb[:], y_acc[:, stq, :], OP.add)
                dma_engs[stq % 3].dma_start(y[stq * 128 : (stq + 1) * 128, :], ysb[:])

    nc.compile()
    return nc


def _get_nc():
    if "nc" not in _cache:
        _cache["nc"] = _build()
    return _cache["nc"]


def _host_prep(query, key, value, Wq, bq, Wk, bk, Wv, bv, Wo, bo):
    """Shard + transpose + cast inputs for the 8 cores."""
    bf = ml_dtypes.bfloat16

    def w_et(W):  # [p, et, dc, i] from W.T[d, e]; d = dc*128+p, e = et*128+i
        WT = np.ascontiguousarray(W.T)
        return np.ascontiguousarray(
            WT.reshape(4, 128, 4, 128).transpose(1, 2, 0, 3)
        ).astype(bf)

    def w_dc(W):  # [p, dc, e]
        WT = np.ascontiguousarray(W.T)
        return np.ascontiguousarray(
            WT.reshape(4, 128, 512).transpose(1, 0, 2)
        ).astype(bf)

    wqr = w_et(Wq)
    wkr = w_et(Wk)
    wvr = w_dc(Wv)
    wor = w_dc(Wo)
    bqr = np.ascontiguousarray(bq.reshape(4, 128).T).astype(np.float32)
    bkr = np.ascontiguousarray(bk.reshape(4, 128).T).astype(np.float32)
    bop = (bo + Wo @ bv).astype(np.float32).reshape(1, D).astype(bf)

    in_maps = []
    for c in range(N_CORES):
        b, half = divmod(c, 2)
        xqT = query[b, half * SQ : (half + 1) * SQ, :].T  # [d, sq]
        xqr = np.ascontiguousarray(
            xqT.reshape(4, 128, 2, 512).transpose(1, 2, 0, 3)
        ).astype(bf)
        keyT = key[b].T  # [d, s]
        keyr = np.ascontiguousarray(
            keyT.reshape(4, 128, 4, 512).transpose(1, 2, 0, 3)
        ).astype(bf)
        valT = value[b].T
        valr = np.ascontiguousarray(
            valT.reshape(4, 128, 16, 128).transpose(1, 2, 0, 3)
        ).astype(bf)
        in_maps.append(
            {
                "xqr": xqr, "keyr": keyr, "valr": valr,
                "wqr": wqr, "wkr": wkr, "wvr": wvr, "wor": wor,
                "bqr": bqr, "bkr": bkr, "bop": bop,
            }
        )
    return in_maps


def _assemble(results):
    out = np.empty((B, S, D), np.float32)
    for c in range(N_CORES):
        b, half = divmod(c, 2)
        out[b, half * SQ : (half + 1) * SQ, :] = results[c]["y"]
    return out


def _run(in_maps, **spmd_kwargs):
    from concourse.bass_utils import run_bass_kernel_spmd

    nc = _get_nc()
    return run_bass_kernel_spmd(nc, in_maps, list(range(N_CORES)), **spmd_kwargs)


def _reference_fallback(query, key, value, mask, Wq, bq, Wk, bk, Wv, bv, Wo, bo):
    """Exact numpy path, used only if the mask is not all-ones."""
    q = (query @ Wq.T + bq).reshape(B, S, H, HD).transpose(0, 2, 1, 3)
    k = (key @ Wk.T + bk).reshape(B, S, H, HD).transpose(0, 2, 1, 3)
    v = (value @ Wv.T + bv).reshape(B, S, H, HD).transpose(0, 2, 1, 3)
    scores = np.einsum("bhqd,bhkd->bhqk", q, k) / np.sqrt(HD).astype(np.float32)
    scores = np.where(mask[:, None, :, :] == 0, -np.inf, scores)
    scores = scores - scores.max(axis=-1, keepdims=True)
    e = np.exp(scores)
    attn = e / e.sum(axis=-1, keepdims=True)
    x = np.einsum("bhqk,bhkd->bhqd", attn, v)
    x = x.transpose(0, 2, 1, 3).reshape(B, S, D)
    return (x @ Wo.T + bo).astype(np.float32)


def kernel(query, key, value, mask, Wq, bq, Wk, bk, Wv, bv, Wo, bo):
    query = np.asarray(query, np.float32)
    key = np.asarray(key, np.float32)
    value = np.asarray(value, np.float32)
    mask_np = np.asarray(mask)
    args = [
        np.asarray(a, np.float32)
        for a in (Wq, bq, Wk, bk, Wv, bv, Wo, bo)
    ]
    if not np.all(mask_np != 0):
        return _reference_fallback(query, key, value, mask_np, *args)
    in_maps = _host_prep(query, key, value, *args)
    res = _run(in_maps, trace=False)
    return _assemble(res.results)


# revision 20
# speedup vs baseline: 1.2941x; 1.2941x over previous
"""Multi-head attention (B=4, S=2048, D=512, H=8) on 8 Trainium2 NeuronCores.

Sharding: core c handles batch b = c//2 and query-half h = c%2 (1024 queries).
Each core computes q = (x_q @ Wq.T + bq)/8 for its queries, k/v projections
for its batch's full 2048 keys, full softmax attention for all 8 heads, and
the output projection for its query rows.  Output rows across cores are
disjoint, so there are no collectives.

The kernel is paced by the Scalar (ACT) engine: 128 exp tiles of [128,1024]
(~1.11us each) are the serial rail (~142us).  Everything else is scheduled
around keeping ACT dense:
 - inputs land via four parallel DMA queues (sync/scalar HWDGE for the q/k
   critical path, gpsimd for v/o, vector for biases) with et/blk-sliced
   weight layouts so the first exp fires ~8us in (vs ~30us serialized);
 - a dummy exp at t=0 prefires the ~2.7us ACT table load;
 - PSUM is statically partitioned (scores 4 banks double-buffered, AV 2,
   softmax denominators 1, projections/misc 1) so projection work never
   steals the score buffers that gate exp;
 - AV consumes exp tiles one kt behind the scores (no head-of-line waits on
   the PE FIFO); the two heads of a pair run col-tiled (64 cols each) in one
   [128,512] accumulator, and the denominators accumulate via four
   concurrent M=1 ones-matmuls into one bank;
 - projections are 4-matmul units spread just-in-time across the pair loops;
 - normalization: denominator rows -> partition-0 via DMA, fast-approx
   reciprocal, PE ones-broadcast to [128,512], one DVE multiply; no PE
   transposes and no DRAM bounce;
 - output projection: pairs 0-2 + bias accumulate into y_acc during pair 3,
   pair 3 is pre-normalized and added in a short tail (single K=128 matmul
   + DVE add per 128-row block), output DMA spread over three queues.
"""

import numpy as np
import ml_dtypes

B = 4
S = 2048
D = 512
H = 8
HD = 64
SQ = 1024  # queries per core
N_CORES = 8

_cache = {}


def _build():
    """Build (once) the SPMD Bass program shared by all 8 cores."""
    import concourse.bacc as bacc
    import concourse.mybir as mybir
    import concourse.tile as tile

    f32 = mybir.dt.float32
    bf16 = mybir.dt.bfloat16
    AF = mybir.ActivationFunctionType
    OP = mybir.AluOpType

    nc = bacc.Bacc("TRN2", target_bir_lowering=False, debug=False)

    # Per-core inputs (pre-sliced / transposed / cast on host).
    xqr = nc.dram_tensor("xqr", [128, 2, 4, 512], bf16, kind="ExternalInput").ap()
    keyr = nc.dram_tensor("keyr", [128, 4, 4, 512], bf16, kind="ExternalInput").ap()
    valr = nc.dram_tensor("valr", [128, 16, 4, 128], bf16, kind="ExternalInput").ap()
    wqr = nc.dram_tensor("wqr", [128, 4, 4, 128], bf16, kind="ExternalInput").ap()
    wkr = nc.dram_tensor("wkr", [128, 4, 4, 128], bf16, kind="ExternalInput").ap()
    wvr = nc.dram_tensor("wvr", [128, 4, 512], bf16, kind="ExternalInput").ap()
    wor = nc.dram_tensor("wor", [128, 4, 512], bf16, kind="ExternalInput").ap()
    bqr = nc.dram_tensor("bqr", [128, 4], f32, kind="ExternalInput").ap()
    bkr = nc.dram_tensor("bkr", [128, 4], f32, kind="ExternalInput").ap()
    bop = nc.dram_tensor("bop", [1, D], bf16, kind="ExternalInput").ap()
    y = nc.dram_tensor("y", [SQ, D], f32, kind="ExternalOutput").ap()

    with tile.TileContext(nc) as tc:
        import contextlib

        with contextlib.ExitStack() as ctx:
            const = ctx.enter_context(tc.tile_pool(name="const", bufs=1))
            io = ctx.enter_context(tc.tile_pool(name="io", bufs=1))
            acts = ctx.enter_context(tc.tile_pool(name="acts", bufs=1))
            expp = ctx.enter_context(tc.tile_pool(name="expp", bufs=8))
            work = ctx.enter_context(tc.tile_pool(name="work", bufs=2))
            dramp = ctx.enter_context(
                tc.tile_pool(name="dramp", bufs=2, space="DRAM")
            )
            psS = ctx.enter_context(tc.tile_pool(name="psS", bufs=2, space="PSUM"))
            psAV = ctx.enter_context(tc.tile_pool(name="psAV", bufs=2, space="PSUM"))
            psDen = ctx.enter_context(tc.tile_pool(name="psDen", bufs=1, space="PSUM"))
            psP = ctx.enter_context(tc.tile_pool(name="psP", bufs=1, space="PSUM"))

            # softmax denominators: partition = qc half, rows = heads
            sums = const.tile([2, H, 512], f32)
            rcp = const.tile([2, 2, 512], f32)

            # ---- SBUF input tiles ------------------------------------------
            wq_sb = io.tile([128, 4, 4, 128], bf16)
            wk_sb = io.tile([128, 4, 4, 128], bf16)
            wv_sb = io.tile([128, 4, 512], bf16)
            wo_sb = io.tile([128, 4, 512], bf16)
            xq_sb = io.tile([128, 2, 4, 512], bf16)
            key_sb = io.tile([128, 4, 4, 512], bf16)
            val_sb = io.tile([128, 16, 4, 128], bf16)
            bq_sb = const.tile([128, 4], f32)
            bk_sb = const.tile([128, 4], f32)
            bop_sb = const.tile([1, D], bf16)

            qT = acts.tile([128, 4, SQ], bf16)  # q^T/8, feature-major
            kT = acts.tile([128, 4, S], bf16)  # k^T, feature-major
            v_sb = acts.tile([128, 16, H, HD], bf16)  # v natural [s, (h d)]
            outT = [
                acts.tile([128, SQ], bf16, name=f"outT{i}") for i in range(4)
            ]
            y_acc = acts.tile([128, 8, 512], bf16)

            # ---- input DMAs over three parallel queues ---------------------
            # Per-ring FIFO order = priority: each ring loads its critical
            # phase-1 slices first, then phase-2 bulk.  (Emitted before any
            # scalar-engine compute so the k-path triggers are not stuck
            # behind the ACT table load.)
            # phase 1 — q path (sync HWDGE ring)
            nc.sync.dma_start(wq_sb[:, 0], wqr[:, 0])
            nc.sync.dma_start(bq_sb[:], bqr[:])
            nc.sync.dma_start(xq_sb[:, 0], xqr[:, 0])
            nc.sync.dma_start(xq_sb[:, 1], xqr[:, 1])
            # phase 1 — k path (scalar HWDGE ring)
            nc.scalar.dma_start(wk_sb[:, 0], wkr[:, 0])
            nc.scalar.dma_start(bk_sb[:], bkr[:])
            nc.scalar.dma_start(key_sb[:, 0], keyr[:, 0])
            nc.scalar.dma_start(key_sb[:, 1], keyr[:, 1])
            # phase 1 — v path (gpsimd SWDGE)
            nc.gpsimd.dma_start(bop_sb[:], bop[:])
            nc.gpsimd.dma_start(wv_sb[:], wvr[:])
            for st in range(4):
                nc.gpsimd.dma_start(val_sb[:, st], valr[:, st])
            # phase 2 — bulk, behind each ring's critical items
            for st in range(4, 10):
                nc.sync.dma_start(val_sb[:, st], valr[:, st])
            for et in (1, 2, 3):
                nc.sync.dma_start(wq_sb[:, et], wqr[:, et])
            nc.scalar.dma_start(key_sb[:, 2], keyr[:, 2])
            nc.scalar.dma_start(key_sb[:, 3], keyr[:, 3])
            for et in (1, 2, 3):
                nc.scalar.dma_start(wk_sb[:, et], wkr[:, et])
            for st in range(10, 16):
                nc.gpsimd.dma_start(val_sb[:, st], valr[:, st])
            nc.gpsimd.dma_start(wo_sb[:], wor[:])

            # ---- constants; dummy exp prefires the ACT table load ----------
            ones_row = const.tile([1, 128], bf16)
            nc.vector.memset(ones_row[:], 1.0)
            ones_col = const.tile([128, 1], bf16)
            nc.vector.memset(ones_col[:], 1.0)
            dum_in = const.tile([1, 16], f32)
            nc.vector.memset(dum_in[:], 0.0)
            dum_out = const.tile([1, 16], bf16)
            nc.scalar.activation(dum_out[:], dum_in[:], AF.Exp)

            # ---- projection units (4 matmuls + one DVE op each) ------------
            def emit_qproj(et, qn):
                ps = psP.tile([128, 512], f32, tag="pp", name=f"psq{et}{qn}")
                for dc in range(4):
                    nc.tensor.matmul(
                        ps[:],
                        lhsT=wq_sb[:, et, dc, :],
                        rhs=xq_sb[:, qn, dc, :],
                        start=(dc == 0),
                        stop=(dc == 3),
                    )
                nc.vector.tensor_scalar(
                    qT[:, et, qn * 512 : (qn + 1) * 512],
                    ps[:],
                    bq_sb[:, et : et + 1],
                    0.125,
                    OP.add,
                    OP.mult,
                )

            def emit_kproj(et, blk):
                ps = psP.tile([128, 512], f32, tag="pp", name=f"psk{et}{blk}")
                for dc in range(4):
                    nc.tensor.matmul(
                        ps[:],
                        lhsT=wk_sb[:, et, dc, :],
                        rhs=key_sb[:, blk, dc, :],
                        start=(dc == 0),
                        stop=(dc == 3),
                    )
                nc.vector.tensor_scalar(
                    kT[:, et, blk * 512 : (blk + 1) * 512],
                    ps[:],
                    bk_sb[:, et : et + 1],
                    None,
                    OP.add,
                )

            def emit_vproj(st):
                ps = psP.tile([128, 512], f32, tag="pp", name=f"psv{st}")
                for dc in range(4):
                    nc.tensor.matmul(
                        ps[:],
                        lhsT=val_sb[:, st, dc, :],
                        rhs=wv_sb[:, dc, :],
                        start=(dc == 0),
                        stop=(dc == 3),
                    )
                nc.vector.tensor_copy(
                    v_sb[:, st], ps[:].rearrange("p (h d) -> p h d", h=H)
                )

            def emit_unit(u):
                kind = u[0]
                if kind == "v":
                    emit_vproj(u[1])
                elif kind == "q":
                    emit_qproj(u[1], u[2])
                elif kind == "k":
                    emit_kproj(u[1], u[2])
                elif kind == "a":
                    emit_stage_a(u[1])

            # ---- attention emitters ----------------------------------------
            # Score tiles are (kt, qn)-major: cols 0-511 = head hh0, cols
            # 512-1023 = head hh1, for query block qn.  Both heads' score
            # matmuls share one tile-slot dependency, so they are emitted
            # adjacently and run row-group-concurrent on the PE; likewise
            # the AV matmuls of both heads pair up col-group-concurrent.
            def emit_scores_exp(hp, kt, qn, exp_tiles):
                st_t = psS.tile([128, SQ], f32, tag="st", name=f"st{hp}_{kt}_{qn}")
                for hh in range(2):
                    nc.tensor.matmul(
                        st_t[:, hh * 512 : (hh + 1) * 512],
                        lhsT=kT[64 * hh : 64 * hh + 64, hp, kt * 128 : (kt + 1) * 128],
                        rhs=qT[64 * hh : 64 * hh + 64, hp, qn * 512 : (qn + 1) * 512],
                        start=True,
                        stop=True,
                        tile_position=(64 * hh, 0),
                    )
                e = expp.tile([128, SQ], bf16, tag="exp", name=f"exp{hp}_{kt}_{qn}")
                exp_tiles[qn][kt] = e
                nc.scalar.activation(e[:], st_t[:], AF.Exp)

            def emit_av(hp, kt, qc, exp_tiles, av):
                e = exp_tiles[qc][kt]
                for hh in range(2):
                    nc.tensor.matmul(
                        av[qc][64 * hh : 64 * hh + 64, :],
                        lhsT=v_sb[:, kt, 2 * hp + hh, :],
                        rhs=e[:, hh * 512 : (hh + 1) * 512],
                        start=(kt == 0),
                        stop=(kt == 15),
                        tile_position=(0, 64 * hh),
                        # two col-tiled groups (one per head) share each bank;
                        # has_written tracking is per-partition, but the sim's
                        # group check is bank-granular — skip it
                        skip_group_check=True,
                    )

            def emit_den(hp, kt, exp_tiles, den):
                for j, (hh, qc) in enumerate(((0, 0), (1, 0), (0, 1), (1, 1))):
                    nc.tensor.matmul(
                        den[32 * j : 32 * j + 1, :],
                        lhsT=ones_col[:, 0:1],
                        rhs=exp_tiles[qc][kt][:, hh * 512 : (hh + 1) * 512],
                        start=(kt == 0),
                        stop=(kt == 15),
                        tile_position=(0, 32 * j),
                        skip_group_check=True,
                    )

            # ---- normalization ---------------------------------------------
            def emit_norm_factors(hp, den_sb):
                # gpsimd queue: a trigger waiting here cannot block ACTIVATEs
                for j, (hh, qc) in enumerate(((0, 0), (1, 0), (0, 1), (1, 1))):
                    nc.gpsimd.dma_start(
                        sums[qc : qc + 1, 2 * hp + hh, :],
                        den_sb[32 * j : 32 * j + 1, :],
                    )
                nc.vector.reciprocal_approx_fast(
                    rcp[:, :, :], sums[:, 2 * hp : 2 * hp + 2, :]
                )
                scr = dramp.tile([2, 2, 512], f32, tag="scr", name=f"scr{hp}")
                nc.sync.dma_start(scr[:], rcp[:])
                return scr

            def emit_norm_apply(hp, avsb, scr):
                # rb rows 0-63 = 1/D_h0 broadcast, 64-127 = 1/D_h1: partition-
                # replicating DMA needs a DRAM source (SBUF APs cannot have a
                # zero partition step), hence the scr bounce
                for qc in range(2):
                    rb = work.tile([128, 512], f32, tag="rb", name=f"rb{hp}{qc}")
                    for hh in range(2):
                        nc.sync.dma_start(
                            rb[64 * hh : 64 * hh + 64, :],
                            scr[qc : qc + 1, hh, :].to_broadcast((64, 512)),
                        )
                    nc.vector.tensor_tensor(
                        outT[hp][:, qc * 512 : (qc + 1) * 512],
                        avsb[:, qc * 512 : (qc + 1) * 512],
                        rb[:],
                        OP.mult,
                    )

            # ---- output projection stage A (pairs 0-2 + bias) --------------
            def emit_stage_a(stq):
                ps = psP.tile([128, 512], f32, tag="pp", name=f"psyA{stq}")
                for c in range(3):
                    nc.tensor.matmul(
                        ps[:],
                        lhsT=outT[c][:, stq * 128 : (stq + 1) * 128],
                        rhs=wo_sb[:, c, :],
                        start=(c == 0),
                        stop=False,
                    )
                nc.tensor.matmul(
                    ps[:], lhsT=ones_row[0:1, :], rhs=bop_sb[:], start=False, stop=True
                )
                nc.vector.tensor_copy(y_acc[:, stq, :], ps[:])

            # ---- projection / stage-A unit schedule ------------------------
            unit_sched = [[[] for _ in range(16)] for _ in range(4)]
            unit_sched[0][0].append(("k", 0, 1))
            for st_ in range(4, 16):
                unit_sched[0][st_ - 3].append(("v", st_))
            unit_sched[0][2].append(("k", 0, 2))
            unit_sched[0][4].append(("k", 0, 3))
            unit_sched[0][10].append(("q", 1, 0))
            unit_sched[0][11].append(("q", 1, 1))
            unit_sched[0][13].append(("k", 1, 0))
            for hpp in (1, 2):
                unit_sched[hpp][0].append(("k", hpp, 1))
                unit_sched[hpp][2].append(("k", hpp, 2))
                unit_sched[hpp][4].append(("k", hpp, 3))
                unit_sched[hpp][6].append(("q", hpp + 1, 0))
                unit_sched[hpp][8].append(("q", hpp + 1, 1))
                unit_sched[hpp][10].append(("k", hpp + 1, 0))
            unit_sched[3][0].append(("k", 3, 1))
            unit_sched[3][1].append(("k", 3, 2))
            unit_sched[3][2].append(("k", 3, 3))
            for i in range(8):
                unit_sched[3][3 + i].append(("a", i))

            # ---- prologue: minimal path to the first exp -------------------
            emit_qproj(0, 0)
            emit_qproj(0, 1)
            emit_kproj(0, 0)
            for st_ in range(4):
                emit_vproj(st_)

            # ---- pair loop -------------------------------------------------
            exp_store = [[[None] * 16, [None] * 16] for _ in range(4)]
            pend_apply = {}
            for hp in range(4):
                av = [
                    psAV.tile([128, 512], f32, tag="av", name=f"av{hp}_{qc}")
                    for qc in range(2)
                ]
                den = psDen.tile([128, 512], f32, tag="den", name=f"den{hp}")
                # only partitions 0/32/64/96 are matmul-written; zero the rest
                # so the pair-end full-tile copy reads initialized memory
                nc.vector.memset(den[:], 0.0)
                exp_tiles = exp_store[hp]
                for kt in range(16):
                    us = list(unit_sched[hp][kt])
                    emit_scores_exp(hp, kt, 0, exp_tiles)
                    if kt > 0:
                        emit_av(hp, kt - 1, 0, exp_tiles, av)
                    if us:
                        emit_unit(us.pop(0))
                    emit_scores_exp(hp, kt, 1, exp_tiles)
                    if kt > 0:
                        emit_av(hp, kt - 1, 1, exp_tiles, av)
                        emit_den(hp, kt - 1, exp_tiles, den)
                    if kt == 2 and (hp - 1) in pend_apply:
                        emit_norm_apply(hp - 1, *pend_apply.pop(hp - 1))
                    for u in us:
                        emit_unit(u)
                # drain kt=15: denominators first so the norm chain starts early
                emit_den(hp, 15, exp_tiles, den)
                den_sb = work.tile([128, 512], f32, tag="den_sb", name=f"densb{hp}")
                nc.vector.tensor_copy(den_sb[:], den[:])
                scr = emit_norm_factors(hp, den_sb)
                for qc in range(2):
                    emit_av(hp, 15, qc, exp_tiles, av)
                avsb = work.tile([128, SQ], f32, tag="avsb", name=f"avsb{hp}")
                for qc in range(2):
                    nc.vector.tensor_copy(avsb[:, qc * 512 : (qc + 1) * 512], av[qc][:])
                pend_apply[hp] = (avsb, scr)

            # ---- tail: normalize pair 3, add its projection, store y -------
            emit_norm_apply(3, *pend_apply.pop(3))
            dma_engs = [nc.sync, nc.scalar, nc.gpsimd]
            # rotate stage-B psum through three pools (5 slots) so matmuls
            # never wait on the DVE add chain
            def psb_tile(stq):
                k = stq % 3
                if k == 0:
                    return psS.tile([128, 512], f32, tag="st", name=f"psyB{stq}")
                if k == 1:
                    return psAV.tile([128, 512], f32, tag="av", name=f"psyB{stq}")
                return psDen.tile([128, 512], f32, tag="den", name=f"psyB{stq}")

            for stq in range(8):
                psb = psb_tile(stq)
                nc.tensor.matmul(
                    psb[:],
                    lhsT=outT[3][:, stq * 128 : (stq + 1) * 128],
                    rhs=wo_sb[:, 3, :],
                    start=True,
                    stop=True,
                )
                ysb = work.tile([128, 512], f32, tag="ysb", name=f"ysb{stq}")
                nc.vector.tensor_tensor(ysb[:], psb[:], y_acc[:, stq, :], OP.add)
                dma_engs[stq % 3].dma_start(y[stq * 128 : (stq + 1) * 128, :], ysb[:])

    nc.compile()
    return nc


def _get_nc():
    if "nc" not in _cache:
        _cache["nc"] = _build()
    return _cache["nc"]


def _host_prep(query, key, value, Wq, bq, Wk, bk, Wv, bv, Wo, bo):
    """Shard + transpose + cast inputs for the 8 cores."""
    bf = ml_dtypes.bfloat16

    def w_et(W):  # [p, et, dc, i] from W.T[d, e]; d = dc*128+p, e = et*128+i
        WT = np.ascontiguousarray(W.T)
        return np.ascontiguousarray(
            WT.reshape(4, 128, 4, 128).transpose(1, 2, 0, 3)
        ).astype(bf)

    def w_dc(W):  # [p, dc, e]
        WT = np.ascontiguousarray(W.T)
        return np.ascontiguousarray(
            WT.reshape(4, 128, 512).transpose(1, 0, 2)
        ).astype(bf)

    wqr = w_et(Wq)
    wkr = w_et(Wk)
    wvr = w_dc(Wv)
    wor = w_dc(Wo)
    bqr = np.ascontiguousarray(bq.reshape(4, 128).T).astype(np.float32)
    bkr = np.ascontiguousarray(bk.reshape(4, 128).T).astype(np.float32)
    bop = (bo + Wo @ bv).astype(np.float32).reshape(1, D).astype(bf)

    in_maps = []
    for c in range(N_CORES):
        b, half = divmod(c, 2)
        xqT = query[b, half * SQ : (half + 1) * SQ, :].T  # [d, sq]
        xqr = np.ascontiguousarray(
            xqT.reshape(4, 128, 2, 512).transpose(1, 2, 0, 3)
        ).astype(bf)
        keyT = key[b].T  # [d, s]
        keyr = np.ascontiguousarray(
            keyT.reshape(4, 128, 4, 512).transpose(1, 2, 0, 3)
        ).astype(bf)
        valT = value[b].T
        valr = np.ascontiguousarray(
            valT.reshape(4, 128, 16, 128).transpose(1, 2, 0, 3)
        ).astype(bf)
        in_maps.append(
            {
                "xqr": xqr, "keyr": keyr, "valr": valr,
                "wqr": wqr, "wkr": wkr, "wvr": wvr, "wor": wor,
                "bqr": bqr, "bkr": bkr, "bop": bop,
            }
        )
    return in_maps


def _assemble(results):
    out = np.empty((B, S, D), np.float32)
    for c in range(N_CORES):
        b, half = divmod(c, 2)
        out[b, half * SQ : (half + 1) * SQ, :] = results[c]["y"]
    return out


def _run(in_maps, **spmd_kwargs):
    from concourse.bass_utils import run_bass_kernel_spmd

    nc = _get_nc()
    return run_bass_kernel_spmd(nc, in_maps, list(range(N_CORES)), **spmd_kwargs)


def _reference_fallback(query, key, value, mask, Wq, bq, Wk, bk, Wv, bv, Wo, bo):
    """Exact numpy path, used only if the mask is not all-ones."""
    q = (query @ Wq.T + bq).reshape(B, S, H, HD).transpose(0, 2, 1, 3)
    k = (key @ Wk.T + bk).reshape(B, S, H, HD).transpose(0, 2, 1, 3)
    v = (value @ Wv.T + bv).reshape(B, S, H, HD).transpose(0, 2, 1, 3)
    scores = np.einsum("bhqd,bhkd->bhqk", q, k) / np.sqrt(HD).astype(np.float32)
    scores = np.where(mask[:, None, :, :] == 0, -np.inf, scores)
    scores = scores - scores.max(axis=-1, keepdims=True)
    e = np.exp(scores)
    attn = e / e.sum(axis=-1, keepdims=True)
    x = np.einsum("bhqk,bhkd->bhqd", attn, v)
    x = x.transpose(0, 2, 1, 3).reshape(B, S, D)
    return (x @ Wo.T + bo).astype(np.float32)


def kernel(query, key, value, mask, Wq, bq, Wk, bk, Wv, bv, Wo, bo):
    query = np.asarray(query, np.float32)
    key = np.asarray(key, np.float32)
    value = np.asarray(value, np.float32)
    mask_np = np.asarray(mask)
    args = [
        np.asarray(a, np.float32)
        for a in (Wq, bq, Wk, bk, Wv, bv, Wo, bo)
    ]
    if not np.all(mask_np != 0):
        return _reference_fallback(query, key, value, mask_np, *args)
    in_maps = _host_prep(query, key, value, *args)
    res = _run(in_maps, trace=False)
    return _assemble(res.results)


# revision 24
# speedup vs baseline: 1.3225x; 1.0220x over previous
"""Multi-head attention (B=4, S=2048, D=512, H=8) on 8 Trainium2 NeuronCores.

Sharding: core c handles batch b = c//2 and query-half h = c%2 (1024 queries).
Each core computes q = (x_q @ Wq.T + bq)/8 for its queries, k/v projections
for its batch's full 2048 keys, full softmax attention for all 8 heads, and
the output projection for its query rows.  Output rows across cores are
disjoint, so there are no collectives.

The kernel is paced by the Scalar (ACT) engine: 128 exp tiles of [128,1024]
(~1.11us each) are the serial rail (~142us).  Everything else is scheduled
around keeping ACT dense:
 - inputs land via four parallel DMA queues (sync/scalar HWDGE for the q/k
   critical path, gpsimd for v/o, vector for biases) with et/blk-sliced
   weight layouts so the first exp fires ~8us in (vs ~30us serialized);
 - a dummy exp at t=0 prefires the ~2.7us ACT table load;
 - PSUM is statically partitioned (scores 4 banks double-buffered, AV 2,
   softmax denominators 1, projections/misc 1) so projection work never
   steals the score buffers that gate exp;
 - AV consumes exp tiles one kt behind the scores (no head-of-line waits on
   the PE FIFO); the two heads of a pair run col-tiled (64 cols each) in one
   [128,512] accumulator, and the denominators accumulate via four
   concurrent M=1 ones-matmuls into one bank;
 - projections are 4-matmul units spread just-in-time across the pair loops;
 - normalization: denominator rows -> partition-0 via DMA, fast-approx
   reciprocal, PE ones-broadcast to [128,512], one DVE multiply; no PE
   transposes and no DRAM bounce;
 - output projection: pairs 0-2 + bias accumulate into y_acc during pair 3,
   pair 3 is pre-normalized and added in a short tail (single K=128 matmul
   + DVE add per 128-row block), output DMA spread over three queues.
"""

import numpy as np
import ml_dtypes

B = 4
S = 2048
D = 512
H = 8
HD = 64
SQ = 1024  # queries per core
N_CORES = 8

_cache = {}


def _build():
    """Build (once) the SPMD Bass program shared by all 8 cores."""
    import concourse.bacc as bacc
    import concourse.mybir as mybir
    import concourse.tile as tile

    f32 = mybir.dt.float32
    bf16 = mybir.dt.bfloat16
    AF = mybir.ActivationFunctionType
    OP = mybir.AluOpType

    nc = bacc.Bacc("TRN2", target_bir_lowering=False, debug=False)

    # Per-core inputs (pre-sliced / transposed / cast on host).
    xqr = nc.dram_tensor("xqr", [128, 2, 4, 512], bf16, kind="ExternalInput").ap()
    keyr = nc.dram_tensor("keyr", [128, 4, 4, 512], bf16, kind="ExternalInput").ap()
    valr = nc.dram_tensor("valr", [128, 16, 4, 128], bf16, kind="ExternalInput").ap()
    wqr = nc.dram_tensor("wqr", [128, 4, 4, 128], bf16, kind="ExternalInput").ap()
    wkr = nc.dram_tensor("wkr", [128, 4, 4, 128], bf16, kind="ExternalInput").ap()
    wvr = nc.dram_tensor("wvr", [128, 4, 512], bf16, kind="ExternalInput").ap()
    wor = nc.dram_tensor("wor", [128, 4, 512], bf16, kind="ExternalInput").ap()
    bqr = nc.dram_tensor("bqr", [128, 4], f32, kind="ExternalInput").ap()
    bkr = nc.dram_tensor("bkr", [128, 4], f32, kind="ExternalInput").ap()
    bop = nc.dram_tensor("bop", [1, D], bf16, kind="ExternalInput").ap()
    y = nc.dram_tensor("y", [SQ, D], f32, kind="ExternalOutput").ap()

    with tile.TileContext(nc) as tc:
        import contextlib

        with contextlib.ExitStack() as ctx:
            const = ctx.enter_context(tc.tile_pool(name="const", bufs=1))
            io = ctx.enter_context(tc.tile_pool(name="io", bufs=1))
            acts = ctx.enter_context(tc.tile_pool(name="acts", bufs=1))
            expp = ctx.enter_context(tc.tile_pool(name="expp", bufs=8))
            work = ctx.enter_context(tc.tile_pool(name="work", bufs=2))
            dramp = ctx.enter_context(
                tc.tile_pool(name="dramp", bufs=2, space="DRAM")
            )
            psS = ctx.enter_context(tc.tile_pool(name="psS", bufs=2, space="PSUM"))
            psAV = ctx.enter_context(tc.tile_pool(name="psAV", bufs=2, space="PSUM"))
            psDen = ctx.enter_context(tc.tile_pool(name="psDen", bufs=1, space="PSUM"))
            psP = ctx.enter_context(tc.tile_pool(name="psP", bufs=1, space="PSUM"))

            # softmax denominators: partition = qc half, rows = heads
            sums = const.tile([2, H, 512], f32)
            rcp = const.tile([2, 2, 512], f32)

            # ---- SBUF input tiles ------------------------------------------
            wq_sb = io.tile([128, 4, 4, 128], bf16)
            wk_sb = io.tile([128, 4, 4, 128], bf16)
            wv_sb = io.tile([128, 4, 512], bf16)
            wo_sb = io.tile([128, 4, 512], bf16)
            xq_sb = io.tile([128, 2, 4, 512], bf16)
            key_sb = io.tile([128, 4, 4, 512], bf16)
            val_sb = io.tile([128, 16, 4, 128], bf16)
            bq_sb = const.tile([128, 4], f32)
            bk_sb = const.tile([128, 4], f32)
            bop_sb = const.tile([1, D], bf16)

            qT = acts.tile([128, 4, SQ], bf16)  # q^T/8, feature-major
            kT = acts.tile([128, 4, S], bf16)  # k^T, feature-major
            v_sb = acts.tile([128, 16, H, HD], bf16)  # v natural [s, (h d)]
            outT = [
                acts.tile([128, SQ], bf16, name=f"outT{i}") for i in range(4)
            ]
            y_acc = acts.tile([128, 8, 512], bf16)

            # ---- input DMAs over three parallel queues ---------------------
            # Per-ring FIFO order = priority: each ring loads its critical
            # phase-1 slices first, then phase-2 bulk.  (Emitted before any
            # scalar-engine compute so the k-path triggers are not stuck
            # behind the ACT table load.)
            # phase 1 — q path (sync HWDGE ring)
            nc.sync.dma_start(wq_sb[:, 0], wqr[:, 0])
            nc.sync.dma_start(bq_sb[:], bqr[:])
            nc.sync.dma_start(xq_sb[:, 0], xqr[:, 0])
            nc.sync.dma_start(xq_sb[:, 1], xqr[:, 1])
            # phase 1 — k path (scalar HWDGE ring)
            nc.scalar.dma_start(wk_sb[:, 0], wkr[:, 0])
            nc.scalar.dma_start(bk_sb[:], bkr[:])
            nc.scalar.dma_start(key_sb[:, 0], keyr[:, 0])
            nc.scalar.dma_start(key_sb[:, 1], keyr[:, 1])
            # phase 1 — v path (gpsimd SWDGE)
            nc.gpsimd.dma_start(bop_sb[:], bop[:])
            nc.gpsimd.dma_start(wv_sb[:], wvr[:])
            for st in range(4):
                nc.gpsimd.dma_start(val_sb[:, st], valr[:, st])
            # ---- constants; dummy exp prefires the ACT table load ----------
            ones_row = const.tile([1, 128], bf16)
            nc.vector.memset(ones_row[:], 1.0)
            ones_col = const.tile([128, 1], bf16)
            nc.vector.memset(ones_col[:], 1.0)
            dum_in = const.tile([1, 16], f32)
            nc.vector.memset(dum_in[:], 0.0)
            dum_out = const.tile([1, 16], bf16)
            nc.scalar.activation(dum_out[:], dum_in[:], AF.Exp)

            # ---- projection units (4 matmuls + one DVE op each) ------------
            def emit_qproj(et, qn):
                ps = psP.tile([128, 512], f32, tag="pp", name=f"psq{et}{qn}")
                for dc in range(4):
                    nc.tensor.matmul(
                        ps[:],
                        lhsT=wq_sb[:, et, dc, :],
                        rhs=xq_sb[:, qn, dc, :],
                        start=(dc == 0),
                        stop=(dc == 3),
                    )
                nc.vector.tensor_scalar(
                    qT[:, et, qn * 512 : (qn + 1) * 512],
                    ps[:],
                    bq_sb[:, et : et + 1],
                    0.125,
                    OP.add,
                    OP.mult,
                )

            def emit_kproj(et, blk):
                ps = psP.tile([128, 512], f32, tag="pp", name=f"psk{et}{blk}")
                for dc in range(4):
                    nc.tensor.matmul(
                        ps[:],
                        lhsT=wk_sb[:, et, dc, :],
                        rhs=key_sb[:, blk, dc, :],
                        start=(dc == 0),
                        stop=(dc == 3),
                    )
                nc.vector.tensor_scalar(
                    kT[:, et, blk * 512 : (blk + 1) * 512],
                    ps[:],
                    bk_sb[:, et : et + 1],
                    None,
                    OP.add,
                )

            def emit_vproj(st):
                ps = psP.tile([128, 512], f32, tag="pp", name=f"psv{st}")
                for dc in range(4):
                    nc.tensor.matmul(
                        ps[:],
                        lhsT=val_sb[:, st, dc, :],
                        rhs=wv_sb[:, dc, :],
                        start=(dc == 0),
                        stop=(dc == 3),
                    )
                nc.vector.tensor_copy(
                    v_sb[:, st], ps[:].rearrange("p (h d) -> p h d", h=H)
                )

            def emit_unit(u):
                kind = u[0]
                if kind == "v":
                    emit_vproj(u[1])
                elif kind == "q":
                    emit_qproj(u[1], u[2])
                elif kind == "k":
                    emit_kproj(u[1], u[2])
                elif kind == "a":
                    emit_stage_a(u[1])

            # ---- attention emitters ----------------------------------------
            # Score tiles are (kt, qn)-major: cols 0-511 = head hh0, cols
            # 512-1023 = head hh1, for query block qn.  Both heads' score
            # matmuls share one tile-slot dependency, so they are emitted
            # adjacently and run row-group-concurrent on the PE; likewise
            # the AV matmuls of both heads pair up col-group-concurrent.
            def emit_scores_exp(hp, kt, qn, exp_tiles):
                st_t = psS.tile([128, SQ], f32, tag="st", name=f"st{hp}_{kt}_{qn}")
                for hh in range(2):
                    nc.tensor.matmul(
                        st_t[:, hh * 512 : (hh + 1) * 512],
                        lhsT=kT[64 * hh : 64 * hh + 64, hp, kt * 128 : (kt + 1) * 128],
                        rhs=qT[64 * hh : 64 * hh + 64, hp, qn * 512 : (qn + 1) * 512],
                        start=True,
                        stop=True,
                        tile_position=(64 * hh, 0),
                    )
                e = expp.tile([128, SQ], bf16, tag="exp", name=f"exp{hp}_{kt}_{qn}")
                exp_tiles[qn][kt] = e
                nc.scalar.activation(e[:], st_t[:], AF.Exp)

            def emit_av(hp, kt, qc, exp_tiles, av):
                e = exp_tiles[qc][kt]
                for hh in range(2):
                    nc.tensor.matmul(
                        av[qc][64 * hh : 64 * hh + 64, :],
                        lhsT=v_sb[:, kt, 2 * hp + hh, :],
                        rhs=e[:, hh * 512 : (hh + 1) * 512],
                        start=(kt == 0),
                        stop=(kt == 15),
                        tile_position=(0, 64 * hh),
                        # two col-tiled groups (one per head) share each bank;
                        # has_written tracking is per-partition, but the sim's
                        # group check is bank-granular — skip it
                        skip_group_check=True,
                    )

            def emit_den(hp, kt, exp_tiles, den):
                for j, (hh, qc) in enumerate(((0, 0), (1, 0), (0, 1), (1, 1))):
                    nc.tensor.matmul(
                        den[32 * j : 32 * j + 1, :],
                        lhsT=ones_col[:, 0:1],
                        rhs=exp_tiles[qc][kt][:, hh * 512 : (hh + 1) * 512],
                        start=(kt == 0),
                        stop=(kt == 15),
                        tile_position=(0, 32 * j),
                        skip_group_check=True,
                    )

            # ---- normalization ---------------------------------------------
            def emit_norm_factors(hp, den_sb, tail=False):
                # gpsimd queue mid-kernel: a waiting trigger there cannot
                # block ACTIVATEs; at the tail use both HWDGE rings instead
                for j, (hh, qc) in enumerate(((0, 0), (1, 0), (0, 1), (1, 1))):
                    eng = (nc.sync if j % 2 == 0 else nc.scalar) if tail else nc.gpsimd
                    eng.dma_start(
                        sums[qc : qc + 1, 2 * hp + hh, :],
                        den_sb[32 * j : 32 * j + 1, :],
                    )
                nc.vector.reciprocal_approx_fast(
                    rcp[:, :, :], sums[:, 2 * hp : 2 * hp + 2, :]
                )
                scr = dramp.tile([2, 2, 512], f32, tag="scr", name=f"scr{hp}")
                nc.sync.dma_start(scr[:], rcp[:])
                return scr

            def emit_norm_apply(hp, avsb, scr, tail=False):
                # rb rows 0-63 = 1/D_h0 broadcast, 64-127 = 1/D_h1: partition-
                # replicating DMA needs a DRAM source (SBUF APs cannot have a
                # zero partition step), hence the scr bounce
                for qc in range(2):
                    rb = work.tile([128, 512], f32, tag="rb", name=f"rb{hp}{qc}")
                    for hh in range(2):
                        eng = (nc.sync if qc == 0 else nc.scalar) if tail else nc.sync
                        eng.dma_start(
                            rb[64 * hh : 64 * hh + 64, :],
                            scr[qc : qc + 1, hh, :].to_broadcast((64, 512)),
                        )
                    nc.vector.tensor_tensor(
                        outT[hp][:, qc * 512 : (qc + 1) * 512],
                        avsb[:, qc * 512 : (qc + 1) * 512],
                        rb[:],
                        OP.mult,
                    )

            # ---- output projection stage A (pairs 0-2 + bias) --------------
            def emit_stage_a(stq):
                ps = psP.tile([128, 512], f32, tag="pp", name=f"psyA{stq}")
                for c in range(3):
                    nc.tensor.matmul(
                        ps[:],
                        lhsT=outT[c][:, stq * 128 : (stq + 1) * 128],
                        rhs=wo_sb[:, c, :],
                        start=(c == 0),
                        stop=False,
                    )
                nc.tensor.matmul(
                    ps[:], lhsT=ones_row[0:1, :], rhs=bop_sb[:], start=False, stop=True
                )
                nc.vector.tensor_copy(y_acc[:, stq, :], ps[:])

            # ---- projection / stage-A unit schedule ------------------------
            unit_sched = [[[] for _ in range(16)] for _ in range(4)]
            unit_sched[0][0].append(("k", 0, 1))
            for st_ in range(4, 16):
                unit_sched[0][st_ - 3].append(("v", st_))
            unit_sched[0][2].append(("k", 0, 2))
            unit_sched[0][4].append(("k", 0, 3))
            unit_sched[0][10].append(("q", 1, 0))
            unit_sched[0][11].append(("q", 1, 1))
            unit_sched[0][13].append(("k", 1, 0))
            for hpp in (1, 2):
                unit_sched[hpp][0].append(("k", hpp, 1))
                unit_sched[hpp][2].append(("k", hpp, 2))
                unit_sched[hpp][4].append(("k", hpp, 3))
                unit_sched[hpp][6].append(("q", hpp + 1, 0))
                unit_sched[hpp][8].append(("q", hpp + 1, 1))
                unit_sched[hpp][10].append(("k", hpp + 1, 0))
            unit_sched[3][0].append(("k", 3, 1))
            unit_sched[3][1].append(("k", 3, 2))
            unit_sched[3][2].append(("k", 3, 3))
            for i in range(8):
                unit_sched[3][3 + i].append(("a", i))

            # ---- prologue: minimal path to the first exp -------------------
            exp_store = [[[None] * 16, [None] * 16] for _ in range(4)]
            emit_qproj(0, 0)
            emit_kproj(0, 0)
            emit_scores_exp(0, 0, 0, exp_store[0])
            emit_qproj(0, 1)
            emit_scores_exp(0, 0, 1, exp_store[0])
            for st_ in range(4):
                emit_vproj(st_)

            # phase-2 bulk DMAs, gated on the last critical load (key blk1)
            # via 1-element WAW writes so they do not steal HBM bandwidth
            # from the critical path.  Ring-FIFO keeps them behind phase 1.
            gate_src = key_sb[0:1, 1, 0, 0:1]
            p2 = []
            for st in range(4, 10):
                p2.append((nc.sync, val_sb[:, st], valr[:, st], val_sb[0:1, st, 0, 0:1]))
            for et in (1, 2, 3):
                p2.append((nc.sync, wq_sb[:, et], wqr[:, et], wq_sb[0:1, et, 0, 0:1]))
            p2.append((nc.scalar, key_sb[:, 2], keyr[:, 2], key_sb[0:1, 2, 0, 0:1]))
            p2.append((nc.scalar, key_sb[:, 3], keyr[:, 3], key_sb[0:1, 3, 0, 0:1]))
            for et in (1, 2, 3):
                p2.append((nc.scalar, wk_sb[:, et], wkr[:, et], wk_sb[0:1, et, 0, 0:1]))
            for st in range(10, 16):
                p2.append((nc.gpsimd, val_sb[:, st], valr[:, st], val_sb[0:1, st, 0, 0:1]))
            p2.append((nc.gpsimd, wo_sb[:], wor[:], wo_sb[0:1, 0, 0:1]))
            for eng, dst, srcd, gate_dst in p2:
                nc.vector.tensor_copy(gate_dst, gate_src)
            for eng, dst, srcd, gate_dst in p2:
                eng.dma_start(dst, srcd)

            # ---- pair loop -------------------------------------------------
            pend_apply = {}
            for hp in range(4):
                av = [
                    psAV.tile([128, 512], f32, tag="av", name=f"av{hp}_{qc}")
                    for qc in range(2)
                ]
                den = psDen.tile([128, 512], f32, tag="den", name=f"den{hp}")
                # only partitions 0/32/64/96 are matmul-written; zero the rest
                # so the pair-end full-tile copy reads initialized memory
                nc.vector.memset(den[:], 0.0)
                exp_tiles = exp_store[hp]
                for kt in range(16):
                    us = list(unit_sched[hp][kt])
                    if not (hp == 0 and kt == 0):  # hp0/kt0 done in prologue
                        emit_scores_exp(hp, kt, 0, exp_tiles)
                    if kt > 0:
                        emit_av(hp, kt - 1, 0, exp_tiles, av)
                    if us:
                        emit_unit(us.pop(0))
                    if not (hp == 0 and kt == 0):
                        emit_scores_exp(hp, kt, 1, exp_tiles)
                    if kt > 0:
                        emit_av(hp, kt - 1, 1, exp_tiles, av)
                        emit_den(hp, kt - 1, exp_tiles, den)
                    if kt == 2 and (hp - 1) in pend_apply:
                        emit_norm_apply(hp - 1, *pend_apply.pop(hp - 1))
                    for u in us:
                        emit_unit(u)
                # drain kt=15: denominators first so the norm chain starts early
                emit_den(hp, 15, exp_tiles, den)
                den_sb = work.tile([128, 512], f32, tag="den_sb", name=f"densb{hp}")
                nc.vector.tensor_copy(den_sb[:], den[:])
                scr = emit_norm_factors(hp, den_sb, tail=(hp == 3))
                for qc in range(2):
                    emit_av(hp, 15, qc, exp_tiles, av)
                avsb = work.tile([128, SQ], f32, tag="avsb", name=f"avsb{hp}")
                for qc in range(2):
                    nc.vector.tensor_copy(avsb[:, qc * 512 : (qc + 1) * 512], av[qc][:])
                pend_apply[hp] = (avsb, scr)

            # ---- tail: normalize pair 3, add its projection, store y -------
            emit_norm_apply(3, *pend_apply.pop(3), tail=True)
            dma_engs = [nc.sync, nc.scalar, nc.gpsimd]
            # rotate stage-B psum through three pools (5 slots) so matmuls
            # never wait on the DVE add chain
            def psb_tile(stq):
                k = stq % 3
                if k == 0:
                    return psS.tile([128, 512], f32, tag="st", name=f"psyB{stq}")
                if k == 1:
                    return psAV.tile([128, 512], f32, tag="av", name=f"psyB{stq}")
                return psDen.tile([128, 512], f32, tag="den", name=f"psyB{stq}")

            for stq in range(8):
                psb = psb_tile(stq)
                nc.tensor.matmul(
                    psb[:],
                    lhsT=outT[3][:, stq * 128 : (stq + 1) * 128],
                    rhs=wo_sb[:, 3, :],
                    start=True,
                    stop=True,
                )
                ysb = work.tile([128, 512], f32, tag="ysb", name=f"ysb{stq}", bufs=8)
                nc.vector.tensor_tensor(ysb[:], psb[:], y_acc[:, stq, :], OP.add)
                dma_engs[stq % 3].dma_start(y[stq * 128 : (stq + 1) * 128, :], ysb[:])

    nc.compile()
    return nc


def _get_nc():
    if "nc" not in _cache:
        _cache["nc"] = _build()
    return _cache["nc"]


def _host_prep(query, key, value, Wq, bq, Wk, bk, Wv, bv, Wo, bo):
    """Shard + transpose + cast inputs for the 8 cores."""
    bf = ml_dtypes.bfloat16

    def w_et(W):  # [p, et, dc, i] from W.T[d, e]; d = dc*128+p, e = et*128+i
        WT = np.ascontiguousarray(W.T)
        return np.ascontiguousarray(
            WT.reshape(4, 128, 4, 128).transpose(1, 2, 0, 3)
        ).astype(bf)

    def w_dc(W):  # [p, dc, e]
        WT = np.ascontiguousarray(W.T)
        return np.ascontiguousarray(
            WT.reshape(4, 128, 512).transpose(1, 0, 2)
        ).astype(bf)

    wqr = w_et(Wq)
    wkr = w_et(Wk)
    wvr = w_dc(Wv)
    wor = w_dc(Wo)
    bqr = np.ascontiguousarray(bq.reshape(4, 128).T).astype(np.float32)
    bkr = np.ascontiguousarray(bk.reshape(4, 128).T).astype(np.float32)
    bop = (bo + Wo @ bv).astype(np.float32).reshape(1, D).astype(bf)

    in_maps = []
    for c in range(N_CORES):
        b, half = divmod(c, 2)
        xqT = query[b, half * SQ : (half + 1) * SQ, :].T  # [d, sq]
        xqr = np.ascontiguousarray(
            xqT.reshape(4, 128, 2, 512).transpose(1, 2, 0, 3)
        ).astype(bf)
        keyT = key[b].T  # [d, s]
        keyr = np.ascontiguousarray(
            keyT.reshape(4, 128, 4, 512).transpose(1, 2, 0, 3)
        ).astype(bf)
        valT = value[b].T
        valr = np.ascontiguousarray(
            valT.reshape(4, 128, 16, 128).transpose(1, 2, 0, 3)
        ).astype(bf)
        in_maps.append(
            {
                "xqr": xqr, "keyr": keyr, "valr": valr,
                "wqr": wqr, "wkr": wkr, "wvr": wvr, "wor": wor,
                "bqr": bqr, "bkr": bkr, "bop": bop,
            }
        )
    return in_maps


def _assemble(results):
    out = np.empty((B, S, D), np.float32)
    for c in range(N_CORES):
        b, half = divmod(c, 2)
        out[b, half * SQ : (half + 1) * SQ, :] = results[c]["y"]
    return out


def _run(in_maps, **spmd_kwargs):
    from concourse.bass_utils import run_bass_kernel_spmd

    nc = _get_nc()
    return run_bass_kernel_spmd(nc, in_maps, list(range(N_CORES)), **spmd_kwargs)


def _reference_fallback(query, key, value, mask, Wq, bq, Wk, bk, Wv, bv, Wo, bo):
    """Exact numpy path, used only if the mask is not all-ones."""
    q = (query @ Wq.T + bq).reshape(B, S, H, HD).transpose(0, 2, 1, 3)
    k = (key @ Wk.T + bk).reshape(B, S, H, HD).transpose(0, 2, 1, 3)
    v = (value @ Wv.T + bv).reshape(B, S, H, HD).transpose(0, 2, 1, 3)
    scores = np.einsum("bhqd,bhkd->bhqk", q, k) / np.sqrt(HD).astype(np.float32)
    scores = np.where(mask[:, None, :, :] == 0, -np.inf, scores)
    scores = scores - scores.max(axis=-1, keepdims=True)
    e = np.exp(scores)
    attn = e / e.sum(axis=-1, keepdims=True)
    x = np.einsum("bhqk,bhkd->bhqd", attn, v)
    x = x.transpose(0, 2, 1, 3).reshape(B, S, D)
    return (x @ Wo.T + bo).astype(np.float32)


def kernel(query, key, value, mask, Wq, bq, Wk, bk, Wv, bv, Wo, bo):
    query = np.asarray(query, np.float32)
    key = np.asarray(key, np.float32)
    value = np.asarray(value, np.float32)
    mask_np = np.asarray(mask)
    args = [
        np.asarray(a, np.float32)
        for a in (Wq, bq, Wk, bk, Wv, bv, Wo, bo)
    ]
    if not np.all(mask_np != 0):
        return _reference_fallback(query, key, value, mask_np, *args)
    in_maps = _host_prep(query, key, value, *args)
    res = _run(in_maps, trace=False)
    return _assemble(res.results)


# revision 28
# speedup vs baseline: 1.3843x; 1.0467x over previous
"""Multi-head attention (B=4, S=2048, D=512, H=8) on 8 Trainium2 NeuronCores.

Sharding: core c handles batch b = c//2 and query-half h = c%2 (1024 queries).
Each core computes q = (x_q @ Wq.T + bq)/8 for its queries, k/v projections
for its batch's full 2048 keys, full softmax attention for all 8 heads, and
the output projection for its query rows.  Output rows across cores are
disjoint, so there are no collectives.

The kernel is paced by the Scalar (ACT) engine: 128 exp tiles of [128,1024]
(~1.11us each) are the serial rail (~142us).  Everything else is scheduled
around keeping ACT dense:
 - inputs land via four parallel DMA queues (sync/scalar HWDGE for the q/k
   critical path, gpsimd for v/o, vector for biases) with et/blk-sliced
   weight layouts so the first exp fires ~8us in (vs ~30us serialized);
 - a dummy exp at t=0 prefires the ~2.7us ACT table load;
 - PSUM is statically partitioned (scores 4 banks double-buffered, AV 2,
   softmax denominators 1, projections/misc 1) so projection work never
   steals the score buffers that gate exp;
 - AV consumes exp tiles one kt behind the scores (no head-of-line waits on
   the PE FIFO); the two heads of a pair run col-tiled (64 cols each) in one
   [128,512] accumulator, and the denominators accumulate via four
   concurrent M=1 ones-matmuls into one bank;
 - projections are 4-matmul units spread just-in-time across the pair loops;
 - normalization: denominator rows -> partition-0 via DMA, fast-approx
   reciprocal, PE ones-broadcast to [128,512], one DVE multiply; no PE
   transposes and no DRAM bounce;
 - output projection: pairs 0-2 + bias accumulate into y_acc during pair 3,
   pair 3 is pre-normalized and added in a short tail (single K=128 matmul
   + DVE add per 128-row block), output DMA spread over three queues.
"""

import numpy as np
import ml_dtypes

B = 4
S = 2048
D = 512
H = 8
HD = 64
SQ = 1024  # queries per core
N_CORES = 8

_cache = {}


def _build():
    """Build (once) the SPMD Bass program shared by all 8 cores."""
    import concourse.bacc as bacc
    import concourse.mybir as mybir
    import concourse.tile as tile

    f32 = mybir.dt.float32
    bf16 = mybir.dt.bfloat16
    AF = mybir.ActivationFunctionType
    OP = mybir.AluOpType

    nc = bacc.Bacc("TRN2", target_bir_lowering=False, debug=False)

    # Per-core inputs (pre-sliced / transposed / cast on host).
    xqr = nc.dram_tensor("xqr", [128, 2, 4, 512], bf16, kind="ExternalInput").ap()
    keyr = nc.dram_tensor("keyr", [128, 4, 4, 512], bf16, kind="ExternalInput").ap()
    valr = nc.dram_tensor("valr", [128, 16, 4, 128], bf16, kind="ExternalInput").ap()
    wqr = nc.dram_tensor("wqr", [128, 4, 4, 128], bf16, kind="ExternalInput").ap()
    wkr = nc.dram_tensor("wkr", [128, 4, 4, 128], bf16, kind="ExternalInput").ap()
    wvr = nc.dram_tensor("wvr", [128, 4, 512], bf16, kind="ExternalInput").ap()
    wor = nc.dram_tensor("wor", [128, 4, 512], bf16, kind="ExternalInput").ap()
    bqr = nc.dram_tensor("bqr", [128, 4], f32, kind="ExternalInput").ap()
    bkr = nc.dram_tensor("bkr", [128, 4], f32, kind="ExternalInput").ap()
    bop = nc.dram_tensor("bop", [1, D], bf16, kind="ExternalInput").ap()
    y = nc.dram_tensor("y", [SQ, D], f32, kind="ExternalOutput").ap()

    with tile.TileContext(nc) as tc:
        import contextlib

        with contextlib.ExitStack() as ctx:
            const = ctx.enter_context(tc.tile_pool(name="const", bufs=1))
            io = ctx.enter_context(tc.tile_pool(name="io", bufs=1))
            acts = ctx.enter_context(tc.tile_pool(name="acts", bufs=1))
            expp = ctx.enter_context(tc.tile_pool(name="expp", bufs=8))
            work = ctx.enter_context(tc.tile_pool(name="work", bufs=2))
            psS = ctx.enter_context(tc.tile_pool(name="psS", bufs=2, space="PSUM"))
            psAV = ctx.enter_context(tc.tile_pool(name="psAV", bufs=2, space="PSUM"))
            psDen = ctx.enter_context(tc.tile_pool(name="psDen", bufs=1, space="PSUM"))
            psP = ctx.enter_context(tc.tile_pool(name="psP", bufs=1, space="PSUM"))

            # softmax denominators: qc halves live on partitions 0 and 32
            # (matmul operands must start on a 32-aligned partition)
            sums = const.tile([33, H, 512], f32)
            rcp = const.tile([33, 2, 512], f32)

            # ---- SBUF input tiles ------------------------------------------
            wq_sb = io.tile([128, 4, 4, 128], bf16)
            wk_sb = io.tile([128, 4, 4, 128], bf16)
            wv_sb = io.tile([128, 4, 512], bf16)
            wo_sb = io.tile([128, 4, 512], bf16)
            xq_sb = io.tile([128, 2, 4, 512], bf16)
            key_sb = io.tile([128, 4, 4, 512], bf16)
            val_sb = io.tile([128, 16, 4, 128], bf16)
            bq_sb = const.tile([128, 4], f32)
            bk_sb = const.tile([128, 4], f32)
            bop_sb = const.tile([1, D], bf16)

            qT = acts.tile([128, 4, SQ], bf16)  # q^T/8, feature-major
            kT = acts.tile([128, 4, S], bf16)  # k^T, feature-major
            v_sb = acts.tile([128, 16, H, HD], bf16)  # v natural [s, (h d)]
            outT = [
                acts.tile([128, SQ], bf16, name=f"outT{i}") for i in range(4)
            ]
            y_acc = acts.tile([128, 8, 512], bf16)

            # ---- input DMAs over three parallel queues ---------------------
            # Per-ring FIFO order = priority: each ring loads its critical
            # phase-1 slices first, then phase-2 bulk.  (Emitted before any
            # scalar-engine compute so the k-path triggers are not stuck
            # behind the ACT table load.)
            # phase 1 — q path (sync HWDGE ring)
            nc.sync.dma_start(wq_sb[:, 0], wqr[:, 0])
            nc.sync.dma_start(bq_sb[:], bqr[:])
            nc.sync.dma_start(xq_sb[:, 0], xqr[:, 0])
            nc.sync.dma_start(xq_sb[:, 1], xqr[:, 1])
            # phase 1 — k path (scalar HWDGE ring)
            nc.scalar.dma_start(wk_sb[:, 0], wkr[:, 0])
            nc.scalar.dma_start(bk_sb[:], bkr[:])
            nc.scalar.dma_start(key_sb[:, 0], keyr[:, 0])
            nc.scalar.dma_start(key_sb[:, 1], keyr[:, 1])
            # phase 1 — v path (gpsimd SWDGE), gated on key blk0 so it does
            # not steal HBM bandwidth from the first-exp critical path
            nc.gpsimd.dma_start(bop_sb[:], bop[:])
            gate0 = key_sb[0:1, 0, 0, 0:1]
            nc.scalar.copy(wv_sb[0:1, 0, 0:1], gate0)
            for st in range(4):
                nc.scalar.copy(val_sb[0:1, st, 0, 0:1], gate0)
            nc.gpsimd.dma_start(wv_sb[:], wvr[:])
            for st in range(4):
                nc.gpsimd.dma_start(val_sb[:, st], valr[:, st])
            # ---- constants; dummy exp prefires the ACT table load ----------
            ones_row = const.tile([1, 128], bf16)
            nc.vector.memset(ones_row[:], 1.0)
            ones_col = const.tile([128, 1], bf16)
            nc.vector.memset(ones_col[:], 1.0)
            ones2 = const.tile([33, 128], bf16)
            nc.vector.memset(ones2[:], 1.0)
            rcp_bf = const.tile([33, 2, 512], bf16)
            nc.vector.memset(sums[:], 1.0)  # keep unused partitions defined
            dum_in = const.tile([1, 16], f32)
            nc.vector.memset(dum_in[:], 0.0)
            dum_out = const.tile([1, 16], bf16)
            nc.scalar.activation(dum_out[:], dum_in[:], AF.Exp)

            # ---- projection units (4 matmuls + one DVE op each) ------------
            def emit_qproj(et, qn, pool=None, tag="pp"):
                ps = (pool or psP).tile([128, 512], f32, tag=tag, name=f"psq{et}{qn}")
                for dc in range(4):
                    nc.tensor.matmul(
                        ps[:],
                        lhsT=wq_sb[:, et, dc, :],
                        rhs=xq_sb[:, qn, dc, :],
                        start=(dc == 0),
                        stop=(dc == 3),
                    )
                nc.vector.tensor_scalar(
                    qT[:, et, qn * 512 : (qn + 1) * 512],
                    ps[:],
                    bq_sb[:, et : et + 1],
                    0.125,
                    OP.add,
                    OP.mult,
                )

            def emit_kproj(et, blk, pool=None, tag="pp"):
                ps = (pool or psP).tile([128, 512], f32, tag=tag, name=f"psk{et}{blk}")
                for dc in range(4):
                    nc.tensor.matmul(
                        ps[:],
                        lhsT=wk_sb[:, et, dc, :],
                        rhs=key_sb[:, blk, dc, :],
                        start=(dc == 0),
                        stop=(dc == 3),
                    )
                nc.vector.tensor_scalar(
                    kT[:, et, blk * 512 : (blk + 1) * 512],
                    ps[:],
                    bk_sb[:, et : et + 1],
                    None,
                    OP.add,
                )

            def emit_vproj(st, pool=None, tag="pp"):
                ps = (pool or psP).tile([128, 512], f32, tag=tag, name=f"psv{st}")
                for dc in range(4):
                    nc.tensor.matmul(
                        ps[:],
                        lhsT=val_sb[:, st, dc, :],
                        rhs=wv_sb[:, dc, :],
                        start=(dc == 0),
                        stop=(dc == 3),
                    )
                nc.vector.tensor_copy(
                    v_sb[:, st], ps[:].rearrange("p (h d) -> p h d", h=H)
                )

            def emit_unit(u):
                kind = u[0]
                if kind == "v":
                    emit_vproj(u[1])
                elif kind == "q":
                    emit_qproj(u[1], u[2])
                elif kind == "k":
                    emit_kproj(u[1], u[2])
                elif kind == "a":
                    emit_stage_a(u[1])

            # ---- attention emitters ----------------------------------------
            # Score tiles are (kt, qn)-major: cols 0-511 = head hh0, cols
            # 512-1023 = head hh1, for query block qn.  Both heads' score
            # matmuls share one tile-slot dependency, so they are emitted
            # adjacently and run row-group-concurrent on the PE; likewise
            # the AV matmuls of both heads pair up col-group-concurrent.
            def emit_scores_exp(hp, kt, qn, exp_tiles):
                st_t = psS.tile([128, SQ], f32, tag="st", name=f"st{hp}_{kt}_{qn}")
                for hh in range(2):
                    nc.tensor.matmul(
                        st_t[:, hh * 512 : (hh + 1) * 512],
                        lhsT=kT[64 * hh : 64 * hh + 64, hp, kt * 128 : (kt + 1) * 128],
                        rhs=qT[64 * hh : 64 * hh + 64, hp, qn * 512 : (qn + 1) * 512],
                        start=True,
                        stop=True,
                        tile_position=(64 * hh, 0),
                    )
                e = expp.tile([128, SQ], bf16, tag="exp", name=f"exp{hp}_{kt}_{qn}")
                exp_tiles[qn][kt] = e
                nc.scalar.activation(e[:], st_t[:], AF.Exp)

            def emit_av(hp, kt, qc, exp_tiles, av):
                e = exp_tiles[qc][kt]
                for hh in range(2):
                    nc.tensor.matmul(
                        av[qc][64 * hh : 64 * hh + 64, :],
                        lhsT=v_sb[:, kt, 2 * hp + hh, :],
                        rhs=e[:, hh * 512 : (hh + 1) * 512],
                        start=(kt == 0),
                        stop=(kt == 15),
                        tile_position=(0, 64 * hh),
                        # two col-tiled groups (one per head) share each bank;
                        # has_written tracking is per-partition, but the sim's
                        # group check is bank-granular — skip it
                        skip_group_check=True,
                    )

            def emit_den(hp, kt, exp_tiles, den):
                for j, (hh, qc) in enumerate(((0, 0), (1, 0), (0, 1), (1, 1))):
                    nc.tensor.matmul(
                        den[32 * j : 32 * j + 1, :],
                        lhsT=ones_col[:, 0:1],
                        rhs=exp_tiles[qc][kt][:, hh * 512 : (hh + 1) * 512],
                        start=(kt == 0),
                        stop=(kt == 15),
                        tile_position=(0, 32 * j),
                        skip_group_check=True,
                    )

            # ---- normalization ---------------------------------------------
            def emit_norm_factors(hp, den_sb, tail=False):
                # gpsimd queue mid-kernel: a waiting trigger there cannot
                # block ACTIVATEs; at the tail use both HWDGE rings instead
                for j, (hh, qc) in enumerate(((0, 0), (1, 0), (0, 1), (1, 1))):
                    eng = (nc.sync if j % 2 == 0 else nc.scalar) if tail else nc.gpsimd
                    eng.dma_start(
                        sums[32 * qc : 32 * qc + 1, 2 * hp + hh, :],
                        den_sb[32 * j : 32 * j + 1, :],
                    )
                nc.vector.reciprocal_approx_fast(
                    rcp[:, :, :], sums[:, 2 * hp : 2 * hp + 2, :]
                )
                nc.vector.tensor_copy(rcp_bf[:], rcp[:])

            def emit_norm_apply(hp, avsb, tail=False):
                # rb rows 0-63 = 1/D_h0 broadcast, 64-127 = 1/D_h1, built by
                # two K=1 ones-matmuls (bf16) — cheaper than a DRAM-bounced
                # replicating DMA chain
                for qc in range(2):
                    rb = psP.tile([128, 512], f32, tag="pp", name=f"rb{hp}{qc}")
                    for hh in range(2):
                        nc.tensor.matmul(
                            rb[64 * hh : 64 * hh + 64, :],
                            lhsT=ones2[32 * qc : 32 * qc + 1, 0:64],
                            rhs=rcp_bf[32 * qc : 32 * qc + 1, hh, :],
                            start=True,
                            stop=True,
                            tile_position=(32 * qc, 64 * hh),
                        )
                    nc.vector.tensor_tensor(
                        outT[hp][:, qc * 512 : (qc + 1) * 512],
                        avsb[:, qc * 512 : (qc + 1) * 512],
                        rb[:],
                        OP.mult,
                    )

            # ---- output projection stage A (pairs 0-2 + bias) --------------
            def emit_stage_a(stq):
                ps = psP.tile([128, 512], f32, tag="pp", name=f"psyA{stq}")
                for c in range(3):
                    nc.tensor.matmul(
                        ps[:],
                        lhsT=outT[c][:, stq * 128 : (stq + 1) * 128],
                        rhs=wo_sb[:, c, :],
                        start=(c == 0),
                        stop=False,
                    )
                nc.tensor.matmul(
                    ps[:], lhsT=ones_row[0:1, :], rhs=bop_sb[:], start=False, stop=True
                )
                nc.vector.tensor_copy(y_acc[:, stq, :], ps[:])

            # ---- projection / stage-A unit schedule ------------------------
            unit_sched = [[[] for _ in range(16)] for _ in range(4)]
            unit_sched[0][0].append(("k", 0, 1))
            for st_ in range(4, 16):
                unit_sched[0][st_ - 3].append(("v", st_))
            unit_sched[0][2].append(("k", 0, 2))
            unit_sched[0][4].append(("k", 0, 3))
            unit_sched[0][10].append(("q", 1, 0))
            unit_sched[0][11].append(("q", 1, 1))
            unit_sched[0][13].append(("k", 1, 0))
            for hpp in (1, 2):
                unit_sched[hpp][0].append(("k", hpp, 1))
                unit_sched[hpp][2].append(("k", hpp, 2))
                unit_sched[hpp][4].append(("k", hpp, 3))
                unit_sched[hpp][6].append(("q", hpp + 1, 0))
                unit_sched[hpp][8].append(("q", hpp + 1, 1))
                unit_sched[hpp][10].append(("k", hpp + 1, 0))
            unit_sched[3][0].append(("k", 3, 1))
            unit_sched[3][1].append(("k", 3, 2))
            unit_sched[3][2].append(("k", 3, 3))
            for i in range(8):
                unit_sched[3][3 + i].append(("a", i))

            # ---- prologue: minimal path to the first exp -------------------
            exp_store = [[[None] * 16, [None] * 16] for _ in range(4)]
            emit_qproj(0, 0)
            emit_kproj(0, 0, psDen, "den")
            emit_scores_exp(0, 0, 0, exp_store[0])
            emit_qproj(0, 1, psAV, "av")
            emit_scores_exp(0, 0, 1, exp_store[0])
            emit_vproj(0, psAV, "av")
            emit_vproj(1)
            emit_vproj(2, psDen, "den")
            emit_vproj(3, psAV, "av")

            # phase-2 bulk DMAs, gated on the last critical load (key blk1)
            # via 1-element WAW writes so they do not steal HBM bandwidth
            # from the critical path.  Ring-FIFO keeps them behind phase 1.
            gate_src = key_sb[0:1, 1, 0, 0:1]
            p2 = []
            p2.append((nc.scalar, key_sb[:, 2], keyr[:, 2], key_sb[0:1, 2, 0, 0:1]))
            p2.append((nc.scalar, key_sb[:, 3], keyr[:, 3], key_sb[0:1, 3, 0, 0:1]))
            for st in range(4, 10):
                p2.append((nc.sync, val_sb[:, st], valr[:, st], val_sb[0:1, st, 0, 0:1]))
            for et in (1, 2, 3):
                p2.append((nc.sync, wq_sb[:, et], wqr[:, et], wq_sb[0:1, et, 0, 0:1]))
            for et in (1, 2, 3):
                p2.append((nc.scalar, wk_sb[:, et], wkr[:, et], wk_sb[0:1, et, 0, 0:1]))
            for st in range(10, 16):
                p2.append((nc.gpsimd, val_sb[:, st], valr[:, st], val_sb[0:1, st, 0, 0:1]))
            p2.append((nc.gpsimd, wo_sb[:], wor[:], wo_sb[0:1, 0, 0:1]))
            for eng, dst, srcd, gate_dst in p2:
                nc.scalar.copy(gate_dst, gate_src)
            for eng, dst, srcd, gate_dst in p2:
                eng.dma_start(dst, srcd)

            # ---- pair loop -------------------------------------------------
            pend_apply = {}
            for hp in range(4):
                av = [
                    psAV.tile([128, 512], f32, tag="av", name=f"av{hp}_{qc}")
                    for qc in range(2)
                ]
                den = psDen.tile([128, 512], f32, tag="den", name=f"den{hp}")
                # only partitions 0/32/64/96 are matmul-written; zero the rest
                # so the pair-end full-tile copy reads initialized memory
                nc.vector.memset(den[:], 0.0)
                exp_tiles = exp_store[hp]
                for kt in range(16):
                    us = list(unit_sched[hp][kt])
                    if not (hp == 0 and kt == 0):  # hp0/kt0 done in prologue
                        emit_scores_exp(hp, kt, 0, exp_tiles)
                    if kt > 0:
                        emit_av(hp, kt - 1, 0, exp_tiles, av)
                    if us:
                        emit_unit(us.pop(0))
                    if not (hp == 0 and kt == 0):
                        emit_scores_exp(hp, kt, 1, exp_tiles)
                    if kt > 0:
                        emit_av(hp, kt - 1, 1, exp_tiles, av)
                        emit_den(hp, kt - 1, exp_tiles, den)
                    if kt == 2 and (hp - 1) in pend_apply:
                        emit_norm_apply(hp - 1, pend_apply.pop(hp - 1))
                    for u in us:
                        emit_unit(u)
                # drain kt=15: denominators first so the norm chain starts early
                emit_den(hp, 15, exp_tiles, den)
                den_sb = work.tile([128, 512], f32, tag="den_sb", name=f"densb{hp}")
                nc.vector.tensor_copy(den_sb[:], den[:])
                emit_norm_factors(hp, den_sb, tail=(hp == 3))
                for qc in range(2):
                    emit_av(hp, 15, qc, exp_tiles, av)
                avsb = work.tile([128, SQ], f32, tag="avsb", name=f"avsb{hp}")
                for qc in range(2):
                    nc.vector.tensor_copy(avsb[:, qc * 512 : (qc + 1) * 512], av[qc][:])
                pend_apply[hp] = avsb

            # ---- tail: normalize pair 3, add its projection, store y -------
            emit_norm_apply(3, pend_apply.pop(3), tail=True)
            dma_engs = [nc.sync, nc.scalar, nc.gpsimd]
            # rotate stage-B psum through three pools (5 slots) so matmuls
            # never wait on the DVE add chain
            def psb_tile(stq):
                k = stq % 3
                if k == 0:
                    return psS.tile([128, 512], f32, tag="st", name=f"psyB{stq}")
                if k == 1:
                    return psAV.tile([128, 512], f32, tag="av", name=f"psyB{stq}")
                return psDen.tile([128, 512], f32, tag="den", name=f"psyB{stq}")

            for stq in range(8):
                psb = psb_tile(stq)
                nc.tensor.matmul(
                    psb[:],
                    lhsT=outT[3][:, stq * 128 : (stq + 1) * 128],
                    rhs=wo_sb[:, 3, :],
                    start=True,
                    stop=True,
                )
                ysb = work.tile([128, 512], f32, tag="ysb", name=f"ysb{stq}", bufs=8)
                nc.vector.tensor_tensor(ysb[:], psb[:], y_acc[:, stq, :], OP.add)
                dma_engs[stq % 3].dma_start(y[stq * 128 : (stq + 1) * 128, :], ysb[:])

    nc.compile()
    return nc


def _get_nc():
    if "nc" not in _cache:
        _cache["nc"] = _build()
    return _cache["nc"]


def _host_prep(query, key, value, Wq, bq, Wk, bk, Wv, bv, Wo, bo):
    """Shard + transpose + cast inputs for the 8 cores."""
    bf = ml_dtypes.bfloat16

    def w_et(W):  # [p, et, dc, i] from W.T[d, e]; d = dc*128+p, e = et*128+i
        WT = np.ascontiguousarray(W.T)
        return np.ascontiguousarray(
            WT.reshape(4, 128, 4, 128).transpose(1, 2, 0, 3)
        ).astype(bf)

    def w_dc(W):  # [p, dc, e]
        WT = np.ascontiguousarray(W.T)
        return np.ascontiguousarray(
            WT.reshape(4, 128, 512).transpose(1, 0, 2)
        ).astype(bf)

    wqr = w_et(Wq)
    wkr = w_et(Wk)
    wvr = w_dc(Wv)
    wor = w_dc(Wo)
    bqr = np.ascontiguousarray(bq.reshape(4, 128).T).astype(np.float32)
    bkr = np.ascontiguousarray(bk.reshape(4, 128).T).astype(np.float32)
    bop = (bo + Wo @ bv).astype(np.float32).reshape(1, D).astype(bf)

    in_maps = []
    for c in range(N_CORES):
        b, half = divmod(c, 2)
        xqT = query[b, half * SQ : (half + 1) * SQ, :].T  # [d, sq]
        xqr = np.ascontiguousarray(
            xqT.reshape(4, 128, 2, 512).transpose(1, 2, 0, 3)
        ).astype(bf)
        keyT = key[b].T  # [d, s]
        keyr = np.ascontiguousarray(
            keyT.reshape(4, 128, 4, 512).transpose(1, 2, 0, 3)
        ).astype(bf)
        valT = value[b].T
        valr = np.ascontiguousarray(
            valT.reshape(4, 128, 16, 128).transpose(1, 2, 0, 3)
        ).astype(bf)
        in_maps.append(
            {
                "xqr": xqr, "keyr": keyr, "valr": valr,
                "wqr": wqr, "wkr": wkr, "wvr": wvr, "wor": wor,
                "bqr": bqr, "bkr": bkr, "bop": bop,
            }
        )
    return in_maps


def _assemble(results):
    out = np.empty((B, S, D), np.float32)
    for c in range(N_CORES):
        b, half = divmod(c, 2)
        out[b, half * SQ : (half + 1) * SQ, :] = results[c]["y"]
    return out


def _run(in_maps, **spmd_kwargs):
    from concourse.bass_utils import run_bass_kernel_spmd

    nc = _get_nc()
    return run_bass_kernel_spmd(nc, in_maps, list(range(N_CORES)), **spmd_kwargs)


def _reference_fallback(query, key, value, mask, Wq, bq, Wk, bk, Wv, bv, Wo, bo):
    """Exact numpy path, used only if the mask is not all-ones."""
    q = (query @ Wq.T + bq).reshape(B, S, H, HD).transpose(0, 2, 1, 3)
    k = (key @ Wk.T + bk).reshape(B, S, H, HD).transpose(0, 2, 1, 3)
    v = (value @ Wv.T + bv).reshape(B, S, H, HD).transpose(0, 2, 1, 3)
    scores = np.einsum("bhqd,bhkd->bhqk", q, k) / np.sqrt(HD).astype(np.float32)
    scores = np.where(mask[:, None, :, :] == 0, -np.inf, scores)
    scores = scores - scores.max(axis=-1, keepdims=True)
    e = np.exp(scores)
    attn = e / e.sum(axis=-1, keepdims=True)
    x = np.einsum("bhqk,bhkd->bhqd", attn, v)
    x = x.transpose(0, 2, 1, 3).reshape(B, S, D)
    return (x @ Wo.T + bo).astype(np.float32)


def kernel(query, key, value, mask, Wq, bq, Wk, bk, Wv, bv, Wo, bo):
    query = np.asarray(query, np.float32)
    key = np.asarray(key, np.float32)
    value = np.asarray(value, np.float32)
    mask_np = np.asarray(mask)
    args = [
        np.asarray(a, np.float32)
        for a in (Wq, bq, Wk, bk, Wv, bv, Wo, bo)
    ]
    if not np.all(mask_np != 0):
        return _reference_fallback(query, key, value, mask_np, *args)
    in_maps = _host_prep(query, key, value, *args)
    res = _run(in_maps, trace=False)
    return _assemble(res.results)


# revision 29
# speedup vs baseline: 1.4459x; 1.0445x over previous
"""Multi-head attention (B=4, S=2048, D=512, H=8) on 8 Trainium2 NeuronCores.

Sharding: core c handles batch b = c//2 and query-half h = c%2 (1024 queries).
Each core computes q = (x_q @ Wq.T + bq)/8 for its queries, k/v projections
for its batch's full 2048 keys, full softmax attention for all 8 heads, and
the output projection for its query rows.  Output rows across cores are
disjoint, so there are no collectives.

The kernel is paced by the Scalar (ACT) engine: 128 exp tiles of [128,1024]
(~1.11us each) are the serial rail (~142us).  Everything else is scheduled
around keeping ACT dense:
 - inputs land via four parallel DMA queues (sync/scalar HWDGE for the q/k
   critical path, gpsimd for v/o, vector for biases) with et/blk-sliced
   weight layouts so the first exp fires ~8us in (vs ~30us serialized);
 - a dummy exp at t=0 prefires the ~2.7us ACT table load;
 - PSUM is statically partitioned (scores 4 banks double-buffered, AV 2,
   softmax denominators 1, projections/misc 1) so projection work never
   steals the score buffers that gate exp;
 - AV consumes exp tiles one kt behind the scores (no head-of-line waits on
   the PE FIFO); the two heads of a pair run col-tiled (64 cols each) in one
   [128,512] accumulator, and the denominators accumulate via four
   concurrent M=1 ones-matmuls into one bank;
 - projections are 4-matmul units spread just-in-time across the pair loops;
 - normalization: denominator rows -> partition-0 via DMA, fast-approx
   reciprocal, PE ones-broadcast to [128,512], one DVE multiply; no PE
   transposes and no DRAM bounce;
 - output projection: pairs 0-2 + bias accumulate into y_acc during pair 3,
   pair 3 is pre-normalized and added in a short tail (single K=128 matmul
   + DVE add per 128-row block), output DMA spread over three queues.
"""

import numpy as np
import ml_dtypes

B = 4
S = 2048
D = 512
H = 8
HD = 64
SQ = 1024  # queries per core
N_CORES = 8

_cache = {}


def _build():
    """Build (once) the SPMD Bass program shared by all 8 cores."""
    import concourse.bacc as bacc
    import concourse.mybir as mybir
    import concourse.tile as tile

    f32 = mybir.dt.float32
    bf16 = mybir.dt.bfloat16
    AF = mybir.ActivationFunctionType
    OP = mybir.AluOpType

    nc = bacc.Bacc("TRN2", target_bir_lowering=False, debug=False)

    # Per-core inputs (pre-sliced / transposed / cast on host).
    xqr = nc.dram_tensor("xqr", [128, 2, 4, 512], bf16, kind="ExternalInput").ap()
    keyr = nc.dram_tensor("keyr", [128, 4, 4, 512], bf16, kind="ExternalInput").ap()
    valr = nc.dram_tensor("valr", [128, 16, 4, 128], bf16, kind="ExternalInput").ap()
    wqr = nc.dram_tensor("wqr", [128, 4, 4, 128], bf16, kind="ExternalInput").ap()
    wkr = nc.dram_tensor("wkr", [128, 4, 4, 128], bf16, kind="ExternalInput").ap()
    wvr = nc.dram_tensor("wvr", [128, 4, 512], bf16, kind="ExternalInput").ap()
    wor = nc.dram_tensor("wor", [128, 4, 512], bf16, kind="ExternalInput").ap()
    bqr = nc.dram_tensor("bqr", [128, 4], f32, kind="ExternalInput").ap()
    bkr = nc.dram_tensor("bkr", [128, 4], f32, kind="ExternalInput").ap()
    bop = nc.dram_tensor("bop", [1, D], bf16, kind="ExternalInput").ap()
    y = nc.dram_tensor("y", [SQ, D], f32, kind="ExternalOutput").ap()

    with tile.TileContext(nc) as tc:
        import contextlib

        with contextlib.ExitStack() as ctx:
            const = ctx.enter_context(tc.tile_pool(name="const", bufs=1))
            io = ctx.enter_context(tc.tile_pool(name="io", bufs=1))
            acts = ctx.enter_context(tc.tile_pool(name="acts", bufs=1))
            expp = ctx.enter_context(tc.tile_pool(name="expp", bufs=10))
            work = ctx.enter_context(tc.tile_pool(name="work", bufs=2))
            psS = ctx.enter_context(tc.tile_pool(name="psS", bufs=2, space="PSUM"))
            psAV = ctx.enter_context(tc.tile_pool(name="psAV", bufs=2, space="PSUM"))
            psDen = ctx.enter_context(tc.tile_pool(name="psDen", bufs=1, space="PSUM"))
            psP = ctx.enter_context(tc.tile_pool(name="psP", bufs=1, space="PSUM"))

            # softmax denominators: qc halves live on partitions 0 and 32
            # (matmul operands must start on a 32-aligned partition)
            sums = const.tile([33, H, 512], f32)
            rcp = const.tile([33, 2, 512], f32)

            # ---- SBUF input tiles ------------------------------------------
            wq_sb = io.tile([128, 4, 4, 128], bf16)
            wk_sb = io.tile([128, 4, 4, 128], bf16)
            wv_sb = io.tile([128, 4, 512], bf16)
            wo_sb = io.tile([128, 4, 512], bf16)
            xq_sb = io.tile([128, 2, 4, 512], bf16)
            key_sb = io.tile([128, 4, 4, 512], bf16)
            val_sb = io.tile([128, 16, 4, 128], bf16)
            bq_sb = const.tile([128, 4], f32)
            bk_sb = const.tile([128, 4], f32)
            bop_sb = const.tile([1, D], bf16)

            qT = acts.tile([128, 4, SQ], bf16)  # q^T/8, feature-major
            kT = acts.tile([128, 4, S], bf16)  # k^T, feature-major
            v_sb = acts.tile([128, 16, H, HD], bf16)  # v natural [s, (h d)]
            outT = [
                acts.tile([128, SQ], bf16, name=f"outT{i}") for i in range(4)
            ]
            y_acc = acts.tile([128, 8, 512], bf16)

            # ---- input DMAs over three parallel queues ---------------------
            # Per-ring FIFO order = priority: each ring loads its critical
            # phase-1 slices first, then phase-2 bulk.  (Emitted before any
            # scalar-engine compute so the k-path triggers are not stuck
            # behind the ACT table load.)
            # phase 1 — q path (sync HWDGE ring)
            nc.sync.dma_start(wq_sb[:, 0], wqr[:, 0])
            nc.sync.dma_start(bq_sb[:], bqr[:])
            nc.sync.dma_start(xq_sb[:, 0], xqr[:, 0])
            nc.sync.dma_start(xq_sb[:, 1], xqr[:, 1])
            # phase 1 — k path (scalar HWDGE ring)
            nc.scalar.dma_start(wk_sb[:, 0], wkr[:, 0])
            nc.scalar.dma_start(bk_sb[:], bkr[:])
            nc.scalar.dma_start(key_sb[:, 0], keyr[:, 0])
            nc.scalar.dma_start(key_sb[:, 1], keyr[:, 1])
            # phase 1 — v path (gpsimd SWDGE), gated on key blk0 so it does
            # not steal HBM bandwidth from the first-exp critical path
            nc.gpsimd.dma_start(bop_sb[:], bop[:])
            gate0 = key_sb[0:1, 0, 0, 0:1]
            nc.scalar.copy(wv_sb[0:1, 0, 0:1], gate0)
            for st in range(4):
                nc.scalar.copy(val_sb[0:1, st, 0, 0:1], gate0)
            nc.gpsimd.dma_start(wv_sb[:], wvr[:])
            for st in range(4):
                nc.gpsimd.dma_start(val_sb[:, st], valr[:, st])
            # ---- constants; dummy exp prefires the ACT table load ----------
            ones_row = const.tile([1, 128], bf16)
            nc.vector.memset(ones_row[:], 1.0)
            ones_col = const.tile([128, 1], bf16)
            nc.vector.memset(ones_col[:], 1.0)
            ones2 = const.tile([33, 128], bf16)
            nc.vector.memset(ones2[:], 1.0)
            rcp_bf = const.tile([33, 2, 512], bf16)
            nc.vector.memset(sums[:], 1.0)  # keep unused partitions defined
            dum_in = const.tile([1, 16], f32)
            nc.vector.memset(dum_in[:], 0.0)
            dum_out = const.tile([1, 16], bf16)
            nc.scalar.activation(dum_out[:], dum_in[:], AF.Exp)

            # ---- projection units (4 matmuls + one DVE op each) ------------
            def emit_qproj(et, qn, pool=None, tag="pp"):
                ps = (pool or psP).tile([128, 512], f32, tag=tag, name=f"psq{et}{qn}")
                for dc in range(4):
                    nc.tensor.matmul(
                        ps[:],
                        lhsT=wq_sb[:, et, dc, :],
                        rhs=xq_sb[:, qn, dc, :],
                        start=(dc == 0),
                        stop=(dc == 3),
                    )
                nc.vector.tensor_scalar(
                    qT[:, et, qn * 512 : (qn + 1) * 512],
                    ps[:],
                    bq_sb[:, et : et + 1],
                    0.125,
                    OP.add,
                    OP.mult,
                )

            def emit_kproj(et, blk, pool=None, tag="pp"):
                ps = (pool or psP).tile([128, 512], f32, tag=tag, name=f"psk{et}{blk}")
                for dc in range(4):
                    nc.tensor.matmul(
                        ps[:],
                        lhsT=wk_sb[:, et, dc, :],
                        rhs=key_sb[:, blk, dc, :],
                        start=(dc == 0),
                        stop=(dc == 3),
                    )
                nc.vector.tensor_scalar(
                    kT[:, et, blk * 512 : (blk + 1) * 512],
                    ps[:],
                    bk_sb[:, et : et + 1],
                    None,
                    OP.add,
                )

            def emit_vproj(st, pool=None, tag="pp"):
                ps = (pool or psP).tile([128, 512], f32, tag=tag, name=f"psv{st}")
                for dc in range(4):
                    nc.tensor.matmul(
                        ps[:],
                        lhsT=val_sb[:, st, dc, :],
                        rhs=wv_sb[:, dc, :],
                        start=(dc == 0),
                        stop=(dc == 3),
                    )
                nc.vector.tensor_copy(
                    v_sb[:, st], ps[:].rearrange("p (h d) -> p h d", h=H)
                )

            def emit_unit(u):
                kind = u[0]
                if kind == "v":
                    emit_vproj(u[1])
                elif kind == "q":
                    emit_qproj(u[1], u[2])
                elif kind == "k":
                    emit_kproj(u[1], u[2])
                elif kind == "a":
                    emit_stage_a(u[1])

            # ---- attention emitters ----------------------------------------
            # Score tiles are (kt, qn)-major: cols 0-511 = head hh0, cols
            # 512-1023 = head hh1, for query block qn.  Both heads' score
            # matmuls share one tile-slot dependency, so they are emitted
            # adjacently and run row-group-concurrent on the PE; likewise
            # the AV matmuls of both heads pair up col-group-concurrent.
            def emit_scores_exp(hp, kt, qn, exp_tiles):
                st_t = psS.tile([128, SQ], f32, tag="st", name=f"st{hp}_{kt}_{qn}")
                for hh in range(2):
                    nc.tensor.matmul(
                        st_t[:, hh * 512 : (hh + 1) * 512],
                        lhsT=kT[64 * hh : 64 * hh + 64, hp, kt * 128 : (kt + 1) * 128],
                        rhs=qT[64 * hh : 64 * hh + 64, hp, qn * 512 : (qn + 1) * 512],
                        start=True,
                        stop=True,
                        tile_position=(64 * hh, 0),
                    )
                e = expp.tile([128, SQ], bf16, tag="exp", name=f"exp{hp}_{kt}_{qn}")
                exp_tiles[qn][kt] = e
                nc.scalar.activation(e[:], st_t[:], AF.Exp)

            def emit_av(hp, kt, qc, exp_tiles, av):
                e = exp_tiles[qc][kt]
                for hh in range(2):
                    nc.tensor.matmul(
                        av[qc][64 * hh : 64 * hh + 64, :],
                        lhsT=v_sb[:, kt, 2 * hp + hh, :],
                        rhs=e[:, hh * 512 : (hh + 1) * 512],
                        start=(kt == 0),
                        stop=(kt == 15),
                        tile_position=(0, 64 * hh),
                        # two col-tiled groups (one per head) share each bank;
                        # has_written tracking is per-partition, but the sim's
                        # group check is bank-granular — skip it
                        skip_group_check=True,
                    )

            def emit_den(hp, kt, exp_tiles, den):
                for j, (hh, qc) in enumerate(((0, 0), (1, 0), (0, 1), (1, 1))):
                    nc.tensor.matmul(
                        den[32 * j : 32 * j + 1, :],
                        lhsT=ones_col[:, 0:1],
                        rhs=exp_tiles[qc][kt][:, hh * 512 : (hh + 1) * 512],
                        start=(kt == 0),
                        stop=(kt == 15),
                        tile_position=(0, 32 * j),
                        skip_group_check=True,
                    )

            # ---- normalization ---------------------------------------------
            def emit_norm_factors(hp, den_sb, tail=False):
                # gpsimd queue mid-kernel: a waiting trigger there cannot
                # block ACTIVATEs; at the tail use both HWDGE rings instead
                for j, (hh, qc) in enumerate(((0, 0), (1, 0), (0, 1), (1, 1))):
                    eng = (nc.sync if j % 2 == 0 else nc.scalar) if tail else nc.gpsimd
                    eng.dma_start(
                        sums[32 * qc : 32 * qc + 1, 2 * hp + hh, :],
                        den_sb[32 * j : 32 * j + 1, :],
                    )
                nc.vector.reciprocal_approx_fast(
                    rcp[:, :, :], sums[:, 2 * hp : 2 * hp + 2, :]
                )
                nc.vector.tensor_copy(rcp_bf[:], rcp[:])

            def emit_norm_apply(hp, avsb, tail=False):
                # rb rows 0-63 = 1/D_h0 broadcast, 64-127 = 1/D_h1, built by
                # two K=1 ones-matmuls (bf16) — cheaper than a DRAM-bounced
                # replicating DMA chain
                for qc in range(2):
                    rb = psP.tile([128, 512], f32, tag="pp", name=f"rb{hp}{qc}")
                    for hh in range(2):
                        nc.tensor.matmul(
                            rb[64 * hh : 64 * hh + 64, :],
                            lhsT=ones2[32 * qc : 32 * qc + 1, 0:64],
                            rhs=rcp_bf[32 * qc : 32 * qc + 1, hh, :],
                            start=True,
                            stop=True,
                            tile_position=(32 * qc, 64 * hh),
                        )
                    nc.vector.tensor_tensor(
                        outT[hp][:, qc * 512 : (qc + 1) * 512],
                        avsb[:, qc * 512 : (qc + 1) * 512],
                        rb[:],
                        OP.mult,
                    )

            # ---- output projection stage A (pairs 0-2 + bias) --------------
            def emit_stage_a(stq):
                ps = psP.tile([128, 512], f32, tag="pp", name=f"psyA{stq}")
                for c in range(3):
                    nc.tensor.matmul(
                        ps[:],
                        lhsT=outT[c][:, stq * 128 : (stq + 1) * 128],
                        rhs=wo_sb[:, c, :],
                        start=(c == 0),
                        stop=False,
                    )
                nc.tensor.matmul(
                    ps[:], lhsT=ones_row[0:1, :], rhs=bop_sb[:], start=False, stop=True
                )
                nc.vector.tensor_copy(y_acc[:, stq, :], ps[:])

            # ---- projection / stage-A unit schedule ------------------------
            unit_sched = [[[] for _ in range(16)] for _ in range(4)]
            unit_sched[0][0].append(("k", 0, 1))
            for st_ in range(4, 16):
                unit_sched[0][st_ - 3].append(("v", st_))
            unit_sched[0][2].append(("k", 0, 2))
            unit_sched[0][4].append(("k", 0, 3))
            unit_sched[0][10].append(("q", 1, 0))
            unit_sched[0][11].append(("q", 1, 1))
            unit_sched[0][13].append(("k", 1, 0))
            for hpp in (1, 2):
                unit_sched[hpp][0].append(("k", hpp, 1))
                unit_sched[hpp][2].append(("k", hpp, 2))
                unit_sched[hpp][4].append(("k", hpp, 3))
                unit_sched[hpp][6].append(("q", hpp + 1, 0))
                unit_sched[hpp][8].append(("q", hpp + 1, 1))
                unit_sched[hpp][10].append(("k", hpp + 1, 0))
            unit_sched[3][0].append(("k", 3, 1))
            unit_sched[3][1].append(("k", 3, 2))
            unit_sched[3][2].append(("k", 3, 3))
            for i in range(8):
                unit_sched[3][5 + i].append(("a", i))

            # ---- prologue: minimal path to the first exp -------------------
            exp_store = [[[None] * 16, [None] * 16] for _ in range(4)]
            emit_qproj(0, 0)
            emit_kproj(0, 0, psDen, "den")
            emit_scores_exp(0, 0, 0, exp_store[0])
            emit_qproj(0, 1, psAV, "av")
            emit_scores_exp(0, 0, 1, exp_store[0])
            emit_vproj(0, psAV, "av")
            emit_vproj(1)
            emit_vproj(2, psDen, "den")
            emit_vproj(3, psAV, "av")

            # phase-2 bulk DMAs, gated on the last critical load (key blk1)
            # via 1-element WAW writes so they do not steal HBM bandwidth
            # from the critical path.  Ring-FIFO keeps them behind phase 1.
            gate_src = key_sb[0:1, 1, 0, 0:1]
            p2 = []
            p2.append((nc.scalar, key_sb[:, 2], keyr[:, 2], key_sb[0:1, 2, 0, 0:1]))
            p2.append((nc.scalar, key_sb[:, 3], keyr[:, 3], key_sb[0:1, 3, 0, 0:1]))
            for st in range(4, 10):
                p2.append((nc.sync, val_sb[:, st], valr[:, st], val_sb[0:1, st, 0, 0:1]))
            for et in (1, 2, 3):
                p2.append((nc.sync, wq_sb[:, et], wqr[:, et], wq_sb[0:1, et, 0, 0:1]))
            for et in (1, 2, 3):
                p2.append((nc.scalar, wk_sb[:, et], wkr[:, et], wk_sb[0:1, et, 0, 0:1]))
            for st in range(10, 16):
                p2.append((nc.gpsimd, val_sb[:, st], valr[:, st], val_sb[0:1, st, 0, 0:1]))
            p2.append((nc.gpsimd, wo_sb[:], wor[:], wo_sb[0:1, 0, 0:1]))
            for eng, dst, srcd, gate_dst in p2:
                nc.scalar.copy(gate_dst, gate_src)
            for eng, dst, srcd, gate_dst in p2:
                eng.dma_start(dst, srcd)

            # ---- pair loop -------------------------------------------------
            pend_apply = {}
            for hp in range(4):
                av = [
                    psAV.tile([128, 512], f32, tag="av", name=f"av{hp}_{qc}")
                    for qc in range(2)
                ]
                den = psDen.tile([128, 512], f32, tag="den", name=f"den{hp}")
                # only partitions 0/32/64/96 are matmul-written; zero the rest
                # so the pair-end full-tile copy reads initialized memory
                nc.vector.memset(den[:], 0.0)
                exp_tiles = exp_store[hp]
                for kt in range(16):
                    us = list(unit_sched[hp][kt])
                    if not (hp == 0 and kt == 0):  # hp0/kt0 done in prologue
                        emit_scores_exp(hp, kt, 0, exp_tiles)
                    if kt > 0:
                        emit_av(hp, kt - 1, 0, exp_tiles, av)
                    if us:
                        emit_unit(us.pop(0))
                    if not (hp == 0 and kt == 0):
                        emit_scores_exp(hp, kt, 1, exp_tiles)
                    if kt > 0:
                        emit_av(hp, kt - 1, 1, exp_tiles, av)
                        emit_den(hp, kt - 1, exp_tiles, den)
                    if kt == 3 and (hp - 1) in pend_apply:
                        emit_norm_apply(hp - 1, pend_apply.pop(hp - 1))
                    for u in us:
                        emit_unit(u)
                # drain kt=15: denominators first so the norm chain starts early
                emit_den(hp, 15, exp_tiles, den)
                den_sb = work.tile([128, 512], f32, tag="den_sb", name=f"densb{hp}")
                nc.vector.tensor_copy(den_sb[:], den[:])
                emit_norm_factors(hp, den_sb, tail=(hp == 3))
                for qc in range(2):
                    emit_av(hp, 15, qc, exp_tiles, av)
                avsb = work.tile([128, SQ], f32, tag="avsb", name=f"avsb{hp}")
                for qc in range(2):
                    nc.vector.tensor_copy(avsb[:, qc * 512 : (qc + 1) * 512], av[qc][:])
                pend_apply[hp] = avsb

            # ---- tail: normalize pair 3, add its projection, store y -------
            emit_norm_apply(3, pend_apply.pop(3), tail=True)
            dma_engs = [nc.sync, nc.scalar, nc.gpsimd]
            # rotate stage-B psum through three pools (5 slots) so matmuls
            # never wait on the DVE add chain
            def psb_tile(stq):
                k = stq % 3
                if k == 0:
                    return psS.tile([128, 512], f32, tag="st", name=f"psyB{stq}")
                if k == 1:
                    return psAV.tile([128, 512], f32, tag="av", name=f"psyB{stq}")
                return psDen.tile([128, 512], f32, tag="den", name=f"psyB{stq}")

            for stq in range(8):
                psb = psb_tile(stq)
                nc.tensor.matmul(
                    psb[:],
                    lhsT=outT[3][:, stq * 128 : (stq + 1) * 128],
                    rhs=wo_sb[:, 3, :],
                    start=True,
                    stop=True,
                )
                ysb = work.tile([128, 512], f32, tag="ysb", name=f"ysb{stq}", bufs=8)
                nc.vector.tensor_tensor(ysb[:], psb[:], y_acc[:, stq, :], OP.add)
                dma_engs[stq % 3].dma_start(y[stq * 128 : (stq + 1) * 128, :], ysb[:])

    nc.compile()
    return nc


def _get_nc():
    if "nc" not in _cache:
        _cache["nc"] = _build()
    return _cache["nc"]


def _host_prep(query, key, value, Wq, bq, Wk, bk, Wv, bv, Wo, bo):
    """Shard + transpose + cast inputs for the 8 cores."""
    bf = ml_dtypes.bfloat16

    def w_et(W):  # [p, et, dc, i] from W.T[d, e]; d = dc*128+p, e = et*128+i
        WT = np.ascontiguousarray(W.T)
        return np.ascontiguousarray(
            WT.reshape(4, 128, 4, 128).transpose(1, 2, 0, 3)
        ).astype(bf)

    def w_dc(W):  # [p, dc, e]
        WT = np.ascontiguousarray(W.T)
        return np.ascontiguousarray(
            WT.reshape(4, 128, 512).transpose(1, 0, 2)
        ).astype(bf)

    wqr = w_et(Wq)
    wkr = w_et(Wk)
    wvr = w_dc(Wv)
    wor = w_dc(Wo)
    bqr = np.ascontiguousarray(bq.reshape(4, 128).T).astype(np.float32)
    bkr = np.ascontiguousarray(bk.reshape(4, 128).T).astype(np.float32)
    bop = (bo + Wo @ bv).astype(np.float32).reshape(1, D).astype(bf)

    in_maps = []
    for c in range(N_CORES):
        b, half = divmod(c, 2)
        xqT = query[b, half * SQ : (half + 1) * SQ, :].T  # [d, sq]
        xqr = np.ascontiguousarray(
            xqT.reshape(4, 128, 2, 512).transpose(1, 2, 0, 3)
        ).astype(bf)
        keyT = key[b].T  # [d, s]
        keyr = np.ascontiguousarray(
            keyT.reshape(4, 128, 4, 512).transpose(1, 2, 0, 3)
        ).astype(bf)
        valT = value[b].T
        valr = np.ascontiguousarray(
            valT.reshape(4, 128, 16, 128).transpose(1, 2, 0, 3)
        ).astype(bf)
        in_maps.append(
            {
                "xqr": xqr, "keyr": keyr, "valr": valr,
                "wqr": wqr, "wkr": wkr, "wvr": wvr, "wor": wor,
                "bqr": bqr, "bkr": bkr, "bop": bop,
            }
        )
    return in_maps


def _assemble(results):
    out = np.empty((B, S, D), np.float32)
    for c in range(N_CORES):
        b, half = divmod(c, 2)
        out[b, half * SQ : (half + 1) * SQ, :] = results[c]["y"]
    return out


def _run(in_maps, **spmd_kwargs):
    from concourse.bass_utils import run_bass_kernel_spmd

    nc = _get_nc()
    return run_bass_kernel_spmd(nc, in_maps, list(range(N_CORES)), **spmd_kwargs)


def _reference_fallback(query, key, value, mask, Wq, bq, Wk, bk, Wv, bv, Wo, bo):
    """Exact numpy path, used only if the mask is not all-ones."""
    q = (query @ Wq.T + bq).reshape(B, S, H, HD).transpose(0, 2, 1, 3)
    k = (key @ Wk.T + bk).reshape(B, S, H, HD).transpose(0, 2, 1, 3)
    v = (value @ Wv.T + bv).reshape(B, S, H, HD).transpose(0, 2, 1, 3)
    scores = np.einsum("bhqd,bhkd->bhqk", q, k) / np.sqrt(HD).astype(np.float32)
    scores = np.where(mask[:, None, :, :] == 0, -np.inf, scores)
    scores = scores - scores.max(axis=-1, keepdims=True)
    e = np.exp(scores)
    attn = e / e.sum(axis=-1, keepdims=True)
    x = np.einsum("bhqk,bhkd->bhqd", attn, v)
    x = x.transpose(0, 2, 1, 3).reshape(B, S, D)
    return (x @ Wo.T + bo).astype(np.float32)


def kernel(query, key, value, mask, Wq, bq, Wk, bk, Wv, bv, Wo, bo):
    query = np.asarray(query, np.float32)
    key = np.asarray(key, np.float32)
    value = np.asarray(value, np.float32)
    mask_np = np.asarray(mask)
    args = [
        np.asarray(a, np.float32)
        for a in (Wq, bq, Wk, bk, Wv, bv, Wo, bo)
    ]
    if not np.all(mask_np != 0):
        return _reference_fallback(query, key, value, mask_np, *args)
    in_maps = _host_prep(query, key, value, *args)
    res = _run(in_maps, trace=False)
    return _assemble(res.results)
